# revision 1
# baseline (speedup 1.0000x reference)
"""Trainium2 Bass kernel for nn_ChunkedCrossAttention_85907935855128.

Self-contained: hardcodes shapes/sharding. Accepts FULL inputs, returns FULL output.
Shards the fused (b*k_chunks) chunk axis across 8 NeuronCores; weights replicated.

Per-core dataflow (all matmul layouts chosen so no on-device transposition of the
big activations is needed; host passes x/context pre-transposed, dim-major):
  qT/kT inner-major via fp32r matmuls (lhsT=W tile, rhs=xT/ctxT), v token-major
  (lhsT=ctxT tile, rhs=Wv). Rope on k = cos*k + sin*(signed-perm matmul on PE).
  Rope on q is identity except each chunk's token 0 (causal shift zeroes the rest
  of the shifted q_pos_emb). Attention in bf16: simT[j,(h,i)] psum -> ACT exp ->
  o[i,65] psum (col 64 = softmax sum via ones column in v_aug) -> reciprocal *
  per-head -> PE-transpose -> fp32r out-projection + bias.
"""
import os
# bass2jax executes via the axon PJRT platform; a CPU pin would hide the cores.
if os.environ.get("JAX_PLATFORMS", "") in ("cpu",):
    del os.environ["JAX_PLATFORMS"]

import numpy as np

import concourse.bacc as bacc
import concourse.bass as bass
import concourse.mybir as mybir
import concourse.tile as tile
from concourse.bass_utils import run_bass_kernel_spmd
from concourse.masks import make_identity

F32 = mybir.dt.float32
F32R = mybir.dt.float32r
BF16 = mybir.dt.bfloat16

CS, CP, H, DH = 64, 63, 8, 64
SCALE = DH ** -0.5
N_CORES = 8
B, N, DIM = 4, 4096, 1024
K_CHUNKS, R, RLEN = 64, 2, 128
TK = R * RLEN                 # 256 ctx tokens / chunk
BK = B * K_CHUNKS             # 256 chunks
CPC = BK // N_CORES           # 32 chunks / core
TQ = CPC * CS                 # 2048 q tokens / core
TCTX = CPC * TK               # 8192 ctx tokens / core
INNER = H * DH                # 512
QG = 4                        # chunks per q-projection group (N=256)
NQG = CPC // QG               # 8 q groups / core


def _build_bass(cpc=CPC, num_devices=N_CORES, do_rope=True, do_attn=True, do_out=True, attn_stop=3):
    tq = cpc * CS
    tctx = cpc * TK
    nqg = cpc // QG
    nc = bacc.Bacc("TRN2", target_bir_lowering=False, debug=False,
                   num_devices=num_devices)

    xT = nc.dram_tensor("xT", (DIM, tq), F32, kind="ExternalInput")
    ctxT = nc.dram_tensor("ctxT", (DIM, tctx), F32, kind="ExternalInput")
    Wq = nc.dram_tensor("Wq", (DIM, INNER), F32, kind="ExternalInput")   # pre-scaled
    Wk = nc.dram_tensor("Wk", (DIM, INNER), F32, kind="ExternalInput")
    Wv = nc.dram_tensor("Wv", (DIM, INNER), F32, kind="ExternalInput")
    Wo = nc.dram_tensor("Wo", (INNER, DIM), F32, kind="ExternalInput")
    bo = nc.dram_tensor("bo", (DIM,), F32, kind="ExternalInput")
    cos_kT = nc.dram_tensor("cos_kT", (64, 128), F32, kind="ExternalInput")
    sin_kT = nc.dram_tensor("sin_kT", (64, 128), F32, kind="ExternalInput")
    Pm = nc.dram_tensor("Pm", (64, 64), F32, kind="ExternalInput")
    nullkT = nc.dram_tensor("nullkT", (64, 8), F32, kind="ExternalInput")
    nullv_aug = nc.dram_tensor("nullv_aug", (1, 8 * 65), F32, kind="ExternalInput")
    cos_q0 = nc.dram_tensor("cos_q0", (64, 1), F32, kind="ExternalInput")
    sin_q0s = nc.dram_tensor("sin_q0s", (64, 1), F32, kind="ExternalInput")
    out = nc.dram_tensor("out", (tq, DIM), F32, kind="ExternalOutput")

    with tile.TileContext(nc) as tc:
        with tc.tile_pool(name="consts", bufs=1) as cp_, \
             tc.tile_pool(name="wk", bufs=2) as wk, \
             tc.tile_pool(name="psb", bufs=3, space="PSUM") as psb, \
             tc.tile_pool(name="pst", bufs=1, space="PSUM") as pst:

            # ---- constants ----
            wq_sb = cp_.tile([128, 8, INNER], F32R)
            nc.sync.dma_start(out=wq_sb, in_=Wq[:, :].rearrange(
                "(dt p) i -> p dt i", p=128).bitcast(F32R))
            wk_sb = cp_.tile([128, 8, INNER], F32R)
            nc.sync.dma_start(out=wk_sb, in_=Wk[:, :].rearrange(
                "(dt p) i -> p dt i", p=128).bitcast(F32R))
            wv_sb = cp_.tile([128, 8, INNER], F32R)
            nc.sync.dma_start(out=wv_sb, in_=Wv[:, :].rearrange(
                "(dt p) i -> p dt i", p=128).bitcast(F32R))
            wo_sb = cp_.tile([128, 4, DIM], F32R)
            nc.sync.dma_start(out=wo_sb, in_=Wo[:, :].rearrange(
                "(et p) c -> p et c", p=128).bitcast(F32R))

            bo_sb = cp_.tile([128, DIM], F32)
            nc.sync.dma_start(out=bo_sb, in_=bass.AP(
                tensor=bo, offset=0, ap=[[0, 128], [1, DIM]]))

            cosk_sb = cp_.tile([64, 128], F32)
            nc.sync.dma_start(out=cosk_sb, in_=cos_kT[:, :])
            sink_sb = cp_.tile([64, 128], F32)
            nc.sync.dma_start(out=sink_sb, in_=sin_kT[:, :])
            cosq_sb = cp_.tile([64, 1], F32)
            nc.sync.dma_start(out=cosq_sb, in_=cos_q0[:, :])
            sinq_sb = cp_.tile([64, 1], F32)
            nc.sync.dma_start(out=sinq_sb, in_=sin_q0s[:, :])

            pm_f32 = cp_.tile([64, 64], F32)
            nc.sync.dma_start(out=pm_f32, in_=Pm[:, :])
            pm_bf = cp_.tile([64, 64], BF16)
            nc.vector.tensor_copy(pm_bf, pm_f32)

            nullk_f32 = cp_.tile([64, 8], F32)
            nc.sync.dma_start(out=nullk_f32, in_=nullkT[:, :])
            nullk_bf = cp_.tile([64, 8], BF16)
            nc.vector.tensor_copy(nullk_bf, nullk_f32)

            nullv_f32 = cp_.tile([1, 8, 65], F32)
            nc.sync.dma_start(out=nullv_f32, in_=nullv_aug[:, :].rearrange(
                "o (h w) -> o h w", h=8))
            nullv_bf = cp_.tile([1, 8, 65], BF16)
            nc.vector.tensor_copy(nullv_bf, nullv_f32)

            ident = cp_.tile([128, 128], F32)
            make_identity(nc, ident)

            for g in range(nqg):          # 8 groups of 4 chunks
                # ---- q projection for this group: qT [512, 256] ----
                xT_sb = wk.tile([128, 8, QG * CS], F32R, tag="xT", bufs=1)
                nc.sync.dma_start(out=xT_sb, in_=xT[:, :].rearrange(
                    "(dt p) t -> p dt t", p=128)[:, :, g * QG * CS:(g + 1) * QG * CS]
                    .bitcast(F32R))
                qps = psb.tile([128, 4, QG * CS], F32, tag="ps", name=f"qps{g}")
                for it in range(4):
                    for dt in range(8):
                        nc.tensor.matmul(
                            qps[:, it, :],
                            wq_sb[:, dt, it * 128:(it + 1) * 128],
                            xT_sb[:, dt, :],
                            start=(dt == 0), stop=(dt == 7))
                qT_sb = wk.tile([64, 8, QG * CS], BF16, tag="qT", bufs=2)
                for it in range(4):
                    nc.vector.tensor_copy(qT_sb[:, 2 * it, :], qps[0:64, it, :])
                    nc.vector.tensor_copy(qT_sb[:, 2 * it + 1, :], qps[64:128, it, :])
                # rope-q: fix token 0 of each chunk (cols ::CS)
                qcols = qT_sb[:, :, :].rearrange(
                    "p h (c w) -> p h c w", w=CS)[:, :, :, 0]   # [64, 8, QG]
                t1q = wk.tile([64, 8, QG], BF16, tag="t1q", bufs=2)
                nc.vector.tensor_mul(
                    t1q, qcols,
                    cosq_sb.unsqueeze(2).broadcast_to((64, 8, QG)))
                t2q = wk.tile([64, 8, QG], BF16, tag="t2q", bufs=2)
                for (dst, src) in ((0, 32), (32, 0)):
                    nc.vector.tensor_mul(
                        t2q[dst:dst + 32, :, :],
                        qT_sb[:, :, :].rearrange(
                            "p h (c w) -> p h c w", w=CS)[src:src + 32, :, :, 0],
                        sinq_sb[src:src + 32, :].unsqueeze(2)
                        .broadcast_to((32, 8, QG)))
                nc.vector.tensor_add(qcols, t1q, t2q)

                # ---- null sims for group: expn_g [1, 8, 256] bf16 ----
                expn_g = wk.tile([1, 8, QG * CS], BF16, tag="expn", bufs=2)
                for h in range(H):
                    nps = pst.tile([1, QG * CS], F32, tag="pst", name=f"nps{g}_{h}")
                    nc.tensor.matmul(
                        nps[:, :],
                        nullk_bf[:, h:h + 1],
                        qT_sb[:, h, :],
                        start=True, stop=True)
                    nc.scalar.activation(expn_g[:, h, :], nps[:, :],
                                         mybir.ActivationFunctionType.Exp)

                for pp in range(QG // 2):
                    cpair = g * QG + pp * 2   # first chunk of the pair
                    # ---- load ctxT pair slice [1024, 512] ----
                    ctx_sb = wk.tile([128, 8, 2 * TK], F32R, tag="ctx", bufs=2)
                    nc.sync.dma_start(out=ctx_sb, in_=ctxT[:, :].rearrange(
                        "(dt p) t -> p dt t", p=128)
                        [:, :, cpair * TK:(cpair + 2) * TK].bitcast(F32R))

                    # ---- k projection for the pair (N=512) ----
                    kps_a = psb.tile([128, 2, 2 * TK], F32, tag="ps", name=f"kpsa{cpair}")
                    kps_b = psb.tile([128, 2, 2 * TK], F32, tag="ps", name=f"kpsb{cpair}")
                    for it in range(4):
                        kp_t = (kps_a, kps_b)[it // 2]
                        for dt in range(8):
                            nc.tensor.matmul(
                                kp_t[:, it % 2, :],
                                wk_sb[:, dt, it * 128:(it + 1) * 128],
                                ctx_sb[:, dt, :],
                                start=(dt == 0), stop=(dt == 7))
                    kraw = wk.tile([64, 8, 2 * TK], BF16, tag="kraw", bufs=2)
                    for it in range(4):
                        kp_t = (kps_a, kps_b)[it // 2]
                        nc.scalar.copy(kraw[:, 2 * it, :], kp_t[0:64, it % 2, :])
                        nc.scalar.copy(kraw[:, 2 * it + 1, :], kp_t[64:128, it % 2, :])

                    # ---- rope-k: perm matmul + combine (pair) ----
                    kpps_t = [
                        psb.tile([64, 2, 2 * TK], F32, tag="ps",
                                 name=f"kpps{q}_{cpair}")
                        for q in range(4)]
                    for q4 in range(8):
                        dst_t = kpps_t[q4 // 2]
                        nc.tensor.matmul(
                            dst_t[:, :, :].rearrange("p h t -> p (h t)")
                            [:, (q4 % 2) * 512:(q4 % 2 + 1) * 512],
                            pm_bf,
                            kraw[:, :, :].rearrange("p h t -> p (h t)")
                            [:, q4 * 512:(q4 + 1) * 512],
                            start=True, stop=True)
                    t1k = wk.tile([64, 8, 2 * TK], BF16, tag="t1k", bufs=1)
                    nc.vector.tensor_mul(
                        t1k[:, :, :].rearrange("p h (rep c) -> p h rep c", rep=4),
                        kraw[:, :, :].rearrange("p h (rep c) -> p h rep c", rep=4),
                        cosk_sb.unsqueeze(1).unsqueeze(2)
                        .broadcast_to((64, 8, 4, 128)))
                    t2k = wk.tile([64, 8, 2 * TK], BF16, tag="t2k", bufs=1)
                    for q, kp_t in enumerate(kpps_t):
                        nc.vector.tensor_mul(
                            t2k[:, q * 2:(q + 1) * 2, :].rearrange(
                                "p h (rep c) -> p h rep c", rep=4),
                            kp_t[:, :, :].rearrange(
                                "p h (rep c) -> p h rep c", rep=4),
                            sink_sb.unsqueeze(1).unsqueeze(2)
                            .broadcast_to((64, 2, 4, 128)))
                    kT_bf = wk.tile([64, 8, 2 * TK], BF16, tag="kT", bufs=2)
                    nc.vector.tensor_add(kT_bf, t1k, t2k)

                    # two chunks of attention per pair
                    for sub in range(2):
                        cc = pp * 2 + sub
                        c = g * QG + cc
                        # ---- v projection -> v_aug bf16 [128, 2, 8, 65] ----
                        vps = psb.tile([128, 2, INNER], F32, tag="ps", name=f"vps{c}")
                        for tg in range(2):
                            for dt in range(8):
                                nc.tensor.matmul(
                                    vps[:, tg, :],
                                    ctx_sb[:, dt, sub * TK + tg * 128:
                                           sub * TK + (tg + 1) * 128],
                                    wv_sb[:, dt, :],
                                    start=(dt == 0), stop=(dt == 7))
                        v_aug = wk.tile([128, 2, 8, 65], BF16, tag="v_aug", bufs=2)
                        nc.scalar.copy(
                            v_aug[:, :, :, 0:64],
                            vps[:, :, :].rearrange("p tg (h w) -> p tg h w", h=8))
                        nc.gpsimd.memset(v_aug[:, :, :, 64:65], 1.0)

                        if not do_attn:
                            continue
                        # ---- sim matmuls: simT [128j, 2jg, (h,i)] ----
                        sps = psb.tile([128, 2, 512], F32, tag="ps", name=f"sps{c}")
                        for h in range(H):
                            for jg in range(2):
                                nc.tensor.matmul(
                                    sps[:, jg, h * 64:(h + 1) * 64],
                                    kT_bf[:, h, sub * TK + jg * 128:
                                          sub * TK + (jg + 1) * 128],
                                    qT_sb[:, h, cc * CS:(cc + 1) * CS],
                                    start=True, stop=True)
                        if attn_stop == 0:
                            dbg = wk.tile([64, DIM], F32, tag="out_sb", bufs=2)
                            nc.vector.tensor_copy(dbg[:, 0:512], sps[0:64, 0, :])
                            nc.vector.memset(dbg[:, 512:], 0.0)
                            nc.sync.dma_start(out=out[c * CS:(c + 1) * CS, :], in_=dbg)
                            continue
                        expT = wk.tile([128, 2, 512], BF16, tag="expT", bufs=2)
                        nc.scalar.activation(expT, sps,
                                             mybir.ActivationFunctionType.Exp)
                        if attn_stop == 1:
                            dbg = wk.tile([64, DIM], F32, tag="out_sb", bufs=2)
                            nc.vector.tensor_copy(dbg[:, 0:512], expT[0:64, 0, :])
                            nc.vector.memset(dbg[:, 512:], 0.0)
                            nc.sync.dma_start(out=out[c * CS:(c + 1) * CS, :], in_=dbg)
                            continue

                        # ---- o matmuls [64i, 65] per head (col 64 = softmax sum) ----
                        ops_ = psb.tile([64, 8, 128], F32, tag="ps", name=f"ops{c}")
                        for h in range(H):
                            dst = ops_[:, h, 0:65]
                            for jg in range(2):
                                nc.tensor.matmul(
                                    dst,
                                    expT[:, jg, h * 64:(h + 1) * 64],
                                    v_aug[:, jg, h, :],
                                    start=(jg == 0), stop=False)
                            nc.tensor.matmul(
                                dst,
                                expn_g[0:1, h, c * CS - g * QG * CS:
                                       c * CS - g * QG * CS + CS],
                                nullv_bf[0:1, h, :],
                                start=False, stop=True)

                        if attn_stop == 2:
                            dbg = wk.tile([64, DIM], F32, tag="out_sb", bufs=2)
                            nc.vector.tensor_copy(dbg[:, 0:128], ops_[:, 0, :])
                            nc.vector.memset(dbg[:, 128:], 0.0)
                            nc.sync.dma_start(out=out[c * CS:(c + 1) * CS, :], in_=dbg)
                            continue
                        # ---- normalize (batched) into pair buffer ----
                        rcol = wk.tile([64, 8], F32, tag="rcol", bufs=2)
                        nc.vector.reciprocal(rcol, ops_[:, :, 64])
                        if sub == 0:
                            o_pair = wk.tile([128, 8, 64], F32, tag="o_pair",
                                             bufs=2)
                        nc.vector.tensor_mul(
                            o_pair[sub * 64:(sub + 1) * 64, :, :],
                            ops_[:, :, 0:64],
                            rcol.unsqueeze(2).broadcast_to((64, 8, 64)))

                        if not do_out:
                            continue
                        if sub == 0:
                            continue
                        # ---- transpose o pair -> oT fp32r [128e, 4et, 128t] ----
                        otr = pst.tile([128, 4, 128], F32, tag="pst",
                                       name=f"otr{cpair}")
                        for et in range(4):
                            nc.tensor.transpose(
                                otr[:, et, :],
                                o_pair[:, 2 * et:2 * et + 2, :],
                                ident)
                        oT_sb = wk.tile([128, 4, 128], F32R, tag="oT", bufs=2)
                        nc.vector.tensor_copy(oT_sb, otr)

                        # ---- out projection + bias (pair, M=128) ----
                        outps = psb.tile([128, DIM], F32, tag="ps",
                                         name=f"outps{cpair}")
                        for co in range(2):
                            for et in range(4):
                                nc.tensor.matmul(
                                    outps[:, co * 512:(co + 1) * 512],
                                    oT_sb[:, et, :],
                                    wo_sb[:, et, co * 512:(co + 1) * 512],
                                    start=(et == 0), stop=(et == 3))
                        out_sb = wk.tile([128, DIM], F32, tag="out_sb", bufs=2)
                        nc.vector.tensor_add(out_sb, outps, bo_sb)
                        nc.sync.dma_start(
                            out=out[cpair * CS:(cpair + 2) * CS, :], in_=out_sb)

    nc.compile()
    return nc


_CACHED_NC = None


def _get_nc():
    global _CACHED_NC
    if _CACHED_NC is None:
        _CACHED_NC = _build_bass()
    return _CACHED_NC


def kernel(x, context, q_pos_emb, k_pos_emb, Wq, Wk, Wv, Wo, bo, null_k, null_v):
    x = np.asarray(x, dtype=np.float32)
    context = np.asarray(context, dtype=np.float32)
    q_pos_emb = np.asarray(q_pos_emb, dtype=np.float32)
    k_pos_emb = np.asarray(k_pos_emb, dtype=np.float32)
    Wq = np.asarray(Wq, dtype=np.float32)
    Wk = np.asarray(Wk, dtype=np.float32)
    Wv = np.asarray(Wv, dtype=np.float32)
    Wo = np.asarray(Wo, dtype=np.float32)
    bo = np.asarray(bo, dtype=np.float32)
    null_k = np.asarray(null_k, dtype=np.float32)
    null_v = np.asarray(null_v, dtype=np.float32)

    # ---- host marshalling (layout only + tiny rope tables) ----
    xs = np.zeros_like(x)
    xs[:, : N - CP] = x[:, CP:]
    xc = xs.reshape(BK, CS, DIM)
    ctx = context.reshape(BK, TK, DIM)

    Wq_s = np.ascontiguousarray(Wq * SCALE)

    qpe63 = q_pos_emb[0, 0, CP]
    cos_q0 = np.cos(qpe63)[:, None].astype(np.float32)          # [64, 1]
    sgn = np.where(np.arange(64) < 32, -1.0, 1.0)
    sin_q0s = (np.sin(qpe63) * sgn)[:, None].astype(np.float32)
    # permuted so the partition-shifted mul reads table at the src base
    # partition (BIR requires equal base partitions for two SBUF inputs)
    sp = np.empty_like(sin_q0s)
    sp[0:32] = sin_q0s[32:64]; sp[32:64] = sin_q0s[0:32]
    sin_q0s = sp

    kpe = k_pos_emb[0, 0]
    cos_kT = np.ascontiguousarray(np.cos(kpe.T).astype(np.float32))   # [64, 128]
    sin_kT = np.ascontiguousarray(np.sin(kpe.T).astype(np.float32))

    Pm = np.zeros((64, 64), np.float32)
    for rout in range(64):
        if rout < 32:
            Pm[rout + 32, rout] = -1.0
        else:
            Pm[rout - 32, rout] = 1.0

    nullkT = np.ascontiguousarray(null_k.reshape(8, 64).T.astype(np.float32))  # [64, 8]
    nullv_aug = np.zeros((1, 8, 65), np.float32)
    nullv_aug[0, :, :64] = null_v.reshape(8, 64)
    nullv_aug[0, :, 64] = 1.0
    nullv_aug = nullv_aug.reshape(1, 8 * 65)

    shared = {
        "Wq": Wq_s, "Wk": Wk, "Wv": Wv, "Wo": Wo, "bo": bo,
        "cos_kT": cos_kT, "sin_kT": sin_kT, "Pm": Pm,
        "nullkT": nullkT, "nullv_aug": nullv_aug,
        "cos_q0": cos_q0, "sin_q0s": sin_q0s,
    }
    in_maps = []
    for c in range(N_CORES):
        sl = slice(c * CPC, (c + 1) * CPC)
        xT_c = np.ascontiguousarray(xc[sl].reshape(TQ, DIM).T)
        ctxT_c = np.ascontiguousarray(ctx[sl].reshape(TCTX, DIM).T)
        in_maps.append({"xT": xT_c, "ctxT": ctxT_c, **shared})

    nc = _get_nc()
    res = run_bass_kernel_spmd(nc, in_maps, core_ids=list(range(N_CORES)))

    out_full = np.concatenate([res.results[c]["out"] for c in range(N_CORES)],
                              axis=0)                      # [BK*CS, DIM]
    o = out_full.reshape(B, K_CHUNKS * CS, DIM)
    final = np.concatenate(
        [np.zeros((B, CP, DIM), np.float32), o[:, : K_CHUNKS * CS - CP]], axis=1)
    return final



# revision 20
# speedup vs baseline: 1.4066x; 1.4066x over previous
"""Trainium2 Bass kernel for nn_ChunkedCrossAttention_85907935855128.

Self-contained: hardcodes shapes/sharding. Accepts FULL inputs, returns FULL
output. Shards the fused (b*k_chunks) chunk axis across 8 NeuronCores.

v2 dataflow per core (32 chunks):
  - k-projection: fp8e4m3 DoubleRow, activation hi+lo error compensation
    (ctx quantized on host to hi+residual fp8 pair; Wk single fp8).
  - v-projection: fully-compensated fp8 DoubleRow ((hi+lo)@(Whi+Wlo)); same
    cost as bf16 but lets ctx live in SBUF as the fp8 pair only.
  - q-projection: fp32r. out-projection: bf16.
  - rope-k on DVE (partition-shifted sin muls, psum scale folded in tables),
    no PE perm matmul.
  - attention: pair-stacked o psum (2 chunks on partition halves), exp bf16
    on ACT, softmax sum via ones column, null-k/v via zero-masked K=2 mms.
"""
import os
# bass2jax executes via the axon PJRT platform; a CPU pin would hide the cores.
if os.environ.get("JAX_PLATFORMS", "") in ("cpu",):
    del os.environ["JAX_PLATFORMS"]

import numpy as np
import ml_dtypes

import concourse.bacc as bacc
import concourse.bass as bass
import concourse.mybir as mybir
import concourse.tile as tile
from concourse.bass_utils import run_bass_kernel_spmd
from concourse.masks import make_identity

F32 = mybir.dt.float32
F32R = mybir.dt.float32r
BF16 = mybir.dt.bfloat16
FP8 = mybir.dt.float8e4
NPF8 = ml_dtypes.float8_e4m3
NPBF = ml_dtypes.bfloat16
DR = mybir.MatmulPerfMode.DoubleRow
EXP = mybir.ActivationFunctionType.Exp
COPY = mybir.ActivationFunctionType.Copy

CS, CP, H, DH = 64, 63, 8, 64
SCALE = DH ** -0.5
N_CORES = 8
B, N, DIM = 4, 4096, 1024
K_CHUNKS, R, RLEN = 64, 2, 128
TK = R * RLEN                 # 256 ctx tokens / chunk
BK = B * K_CHUNKS             # 256 chunks
CPC = BK // N_CORES           # 32 chunks / core
TQ = CPC * CS                 # 2048 q tokens / core
TCTX = CPC * TK               # 8192 ctx tokens / core
INNER = H * DH                # 512
QG = 4                        # chunks per q-projection group
NQG = CPC // QG               # 8 q groups / core

SC_CTX = 3                    # ctx * 2^3 before fp8
SC_WK = 9                     # Wk * 2^9
SC_WV = 9
PSUM_SC = 2.0 ** -(SC_CTX + SC_WK)   # folded into rope tables / v evac


def _build_bass(num_devices=N_CORES, stage=4):
    nc = bacc.Bacc("TRN2", target_bir_lowering=False, debug=False,
                   num_devices=num_devices)

    xT = nc.dram_tensor("xT", (DIM, TQ), F32, kind="ExternalInput")
    ctx8 = nc.dram_tensor("ctx8", (2, DIM, TCTX), FP8, kind="ExternalInput")
    wq = nc.dram_tensor("wq", (128, 8, INNER), F32, kind="ExternalInput")
    wk8 = nc.dram_tensor("wk8", (128, 8, 2, INNER), FP8, kind="ExternalInput")
    wv8 = nc.dram_tensor("wv8", (128, 8, 2, 2, INNER), FP8, kind="ExternalInput")
    wo = nc.dram_tensor("wo", (128, 4, DIM), BF16, kind="ExternalInput")
    bo = nc.dram_tensor("bo", (DIM,), F32, kind="ExternalInput")
    cosk = nc.dram_tensor("cosk", (128, 128), BF16, kind="ExternalInput")
    sink2 = nc.dram_tensor("sink2", (128, 128), BF16, kind="ExternalInput")
    cos_q0 = nc.dram_tensor("cos_q0", (128, 1), F32, kind="ExternalInput")
    sin_q0s = nc.dram_tensor("sin_q0s", (128, 1), F32, kind="ExternalInput")
    nullk2 = nc.dram_tensor("nullk2", (128, 4, 2), BF16, kind="ExternalInput")
    nullv2 = nc.dram_tensor("nullv2", (2, 4, 2, 65), BF16, kind="ExternalInput")
    out = nc.dram_tensor("out", (TQ, DIM), F32, kind="ExternalOutput")

    with tile.TileContext(nc) as tc:
        with tc.tile_pool(name="consts", bufs=1) as cp_, \
             tc.tile_pool(name="wk", bufs=2) as wkp, \
             tc.tile_pool(name="ps", bufs=2, space="PSUM") as psp:

            # ---- constants (ordered by first use) ----
            wq_sb = cp_.tile([128, 8, INNER], F32R)
            nc.sync.dma_start(out=wq_sb, in_=wq[:, :, :].bitcast(F32R))
            wk_sb = cp_.tile([128, 8, 2, INNER], FP8)
            nc.sync.dma_start(out=wk_sb, in_=wk8[:, :, :, :])
            wv_sb = cp_.tile([128, 8, 2, 2, INNER], FP8)
            nc.sync.dma_start(out=wv_sb, in_=wv8[:, :, :, :, :])
            wo_sb = cp_.tile([128, 4, DIM], BF16)
            nc.sync.dma_start(out=wo_sb, in_=wo[:, :, :])
            bo_sb = cp_.tile([128, DIM], F32)
            nc.sync.dma_start(out=bo_sb, in_=bass.AP(
                tensor=bo, offset=0, ap=[[0, 128], [1, DIM]]))
            cosk_sb = cp_.tile([128, 128], BF16)
            nc.sync.dma_start(out=cosk_sb, in_=cosk[:, :])
            sink2_sb = cp_.tile([128, 128], BF16)
            nc.sync.dma_start(out=sink2_sb, in_=sink2[:, :])
            cosq_sb = cp_.tile([128, 1], F32)
            nc.sync.dma_start(out=cosq_sb, in_=cos_q0[:, :])
            sinq_sb = cp_.tile([128, 1], F32)
            nc.sync.dma_start(out=sinq_sb, in_=sin_q0s[:, :])
            nullk_sb = cp_.tile([128, 4, 2], BF16)
            nc.sync.dma_start(out=nullk_sb, in_=nullk2[:, :, :])
            nullv_sb = cp_.tile([2, 4, 2, 65], BF16)
            nc.sync.dma_start(out=nullv_sb, in_=nullv2[:, :, :, :])
            ident = cp_.tile([128, 128], BF16)
            make_identity(nc, ident)

            for g in range(NQG):          # 8 groups of 4 chunks
                gcols = slice(g * QG * CS, (g + 1) * QG * CS)
                # ---- q projection (fp32r): qps [128=2 heads, 2, 256] x2 ----
                xT_sb = wkp.tile([128, 8, QG * CS], F32R, tag="xT")
                nc.sync.dma_start(out=xT_sb, in_=xT[:, :].rearrange(
                    "(dt p) t -> p dt t", p=128)[:, :, gcols].bitcast(F32R))
                qps = [psp.tile([128, 2, QG * CS], F32, tag="p2",
                                name=f"qps{g}_{i}") for i in range(2)]
                for it in range(4):
                    for dt in range(8):
                        nc.tensor.matmul(
                            qps[it // 2][:, it % 2, :],
                            wq_sb[:, dt, it * 128:(it + 1) * 128],
                            xT_sb[:, dt, :],
                            start=(dt == 0), stop=(dt == 7))
                # qT [128 = head-pair dh, 4 hp, 256] bf16
                qT = wkp.tile([128, 4, QG * CS], BF16, tag="qT")
                for it in range(4):
                    nc.scalar.copy(qT[:, it, :], qps[it // 2][:, it % 2, :])
                # rope-q: fix token 0 of each chunk (cols ::CS), both halves
                qcols = qT[:, :, :].rearrange(
                    "p h (c w) -> p h c w", w=CS)[:, :, :, 0]   # [128, 4, QG]
                t1q = wkp.tile([128, 4, QG], BF16, tag="t1q")
                nc.vector.tensor_mul(
                    t1q, qcols, cosq_sb.unsqueeze(2).broadcast_to((128, 4, QG)))
                t2q = wkp.tile([128, 4, QG], BF16, tag="t2q")
                for (dst, src) in ((0, 32), (32, 0), (64, 96), (96, 64)):
                    nc.vector.tensor_mul(
                        t2q[dst:dst + 32, :, :],
                        qT[:, :, :].rearrange(
                            "p h (c w) -> p h c w", w=CS)[src:src + 32, :, :, 0],
                        sinq_sb[src:src + 32, :].unsqueeze(2)
                        .broadcast_to((32, 4, QG)))
                nc.vector.tensor_add(qcols, t1q, t2q)
                # odd heads live on partitions 64-127; PE operands must sit at
                # base 0 on hw, so keep a base-0 copy for the sim matmuls
                qTh = wkp.tile([64, 4, QG * CS], BF16, tag="qTh")
                nc.vector.tensor_copy(qTh, qT[64:128, :, :])

                # ---- null-k sims -> expn bf16 [2, 4 hp, 256] ----
                expn = wkp.tile([2, 4, QG * CS], BF16, tag="expn")
                for half in range(2):
                    nps = psp.tile([2, 2, QG * CS], F32, tag="p2",
                                   name=f"nps{g}_{half}")
                    for hh in range(2):
                        hp = half * 2 + hh
                        nc.tensor.matmul(
                            nps[:, hh, :], nullk_sb[:, hp, :], qT[:, hp, :],
                            start=True, stop=True)
                    nc.scalar.activation(
                        expn[:, half * 2:(half + 1) * 2, :], nps, EXP)

                for pp in range(2):       # pairs within the group
                    pr = g * 2 + pp       # global pair index
                    pcols = slice(pr * 2 * TK, (pr + 1) * 2 * TK)
                    # ---- ctx fp8 pair [128, 8 dt, 2 hi/lo, 512 tok] ----
                    ctx_sb = wkp.tile([128, 8, 2, 2 * TK], FP8, tag="ctx")
                    for s in range(2):
                        nc.sync.dma_start(
                            out=ctx_sb[:, :, s, :],
                            in_=ctx8[s, :, :].rearrange(
                                "(dt p) t -> p dt t", p=128)[:, :, pcols])

                    # ---- k projection (fp8 DR, act-compensated) ----
                    # kraw [128 = head-pair dh, 4 hp, 512] bf16
                    kraw = wkp.tile([128, 4, 2 * TK], BF16, tag="kraw")
                    for it in range(4):
                        kps = psp.tile([128, 2 * TK], F32, tag="p2",
                                       name=f"kps{pr}_{it}")
                        for dt in range(8):
                            nc.tensor.matmul(
                                kps, wk_sb[:, dt, :, it * 128:(it + 1) * 128],
                                ctx_sb[:, dt, :, :],
                                start=(dt == 0), stop=(dt == 7),
                                perf_mode=DR)
                        nc.scalar.copy(kraw[:, it, :], kps)
                    # ---- rope-k on DVE: kT2 = cos*k + sin_shift*k_shift ----
                    t1k = wkp.tile([128, 4, 2 * TK], BF16, tag="t1k")
                    nc.vector.tensor_mul(
                        t1k[:, :, :].rearrange("p h (r c) -> p h r c", r=4),
                        kraw[:, :, :].rearrange("p h (r c) -> p h r c", r=4),
                        cosk_sb.unsqueeze(1).unsqueeze(2)
                        .broadcast_to((128, 4, 4, 128)))
                    t2k = wkp.tile([128, 4, 2 * TK], BF16, tag="t2k")
                    for (dst, src) in ((0, 32), (32, 0), (64, 96), (96, 64)):
                        nc.vector.tensor_mul(
                            t2k[dst:dst + 32, :, :].rearrange(
                                "p h (r c) -> p h r c", r=4),
                            kraw[src:src + 32, :, :].rearrange(
                                "p h (r c) -> p h r c", r=4),
                            sink2_sb[src:src + 32, :].unsqueeze(1).unsqueeze(2)
                            .broadcast_to((32, 4, 4, 128)))
                    kT2 = wkp.tile([128, 4, 2 * TK], BF16, tag="kT2")
                    nc.vector.tensor_add(kT2, t1k, t2k)
                    kT2h = wkp.tile([64, 4, 2 * TK], BF16, tag="kT2h")
                    nc.vector.tensor_copy(kT2h, kT2[64:128, :, :])

                    if stage <= 1:
                        dbg = wkp.tile([128, DIM], F32, tag="out_sb")
                        nc.vector.tensor_copy(dbg[:, 0:512], kT2[:, 0, :])
                        nc.vector.memset(dbg[:, 512:], 0.0)
                        nc.sync.dma_start(
                            out=out[pr * 2 * CS:(pr + 1) * 2 * CS, :], in_=dbg)
                        continue

                    # ---- per-pair o psum [128 = 2 chunks, 8 h, 128] ----
                    ops_ = None
                    if stage in (3, 4):
                        ops_ = psp.tile([128, 8, 128], F32, tag="p4",
                                        name=f"ops{pr}")
                    for sub in range(2):
                        c = pr * 2 + sub            # global chunk
                        cc = pp * 2 + sub           # chunk within group
                        # ---- v projection (fp8 DR, fully compensated) ----
                        v_aug = wkp.tile([128, 2, 8, 65], BF16, tag="v_aug")
                        if stage == 17:      # skip v-proj, dummy v_aug
                            nc.vector.memset(v_aug[:, :, :, :], 1.0)
                        for tg in range(2 if stage != 17 else 0):
                            vps = psp.tile([128, INNER], F32, tag="pv",
                                           name=f"vps{c}_{tg}")
                            tsl = slice(sub * TK + tg * 128,
                                        sub * TK + (tg + 1) * 128)
                            for dt in range(8):
                                for w in range(2):
                                    nc.tensor.matmul(
                                        vps,
                                        ctx_sb[:, dt, :, tsl],
                                        wv_sb[:, dt, w, :, :],
                                        start=(dt == 0 and w == 0),
                                        stop=(dt == 7 and w == 1),
                                        perf_mode=DR)
                            nc.scalar.activation(
                                v_aug[:, tg, :, 0:64],
                                vps[:, :].rearrange("p (h w) -> p h w", h=8),
                                COPY, scale=PSUM_SC)
                        nc.gpsimd.memset(v_aug[:, :, :, 64:65], 1.0)
                        if stage == 15:      # v-proj only
                            if sub == 1:
                                continue
                            dbg = wkp.tile([128, DIM], F32, tag="out_sb")
                            nc.vector.tensor_copy(
                                dbg[:, 0:512].rearrange(
                                    "p (h w) -> p h w", h=8),
                                v_aug[:, 0, :, 0:64])
                            nc.vector.memset(dbg[:, 512:], 0.0)
                            nc.sync.dma_start(
                                out=out[pr * 2 * CS:(pr + 1) * 2 * CS, :],
                                in_=dbg)
                            continue

                        # ---- sim [128 j, 2 jg, (h, i)] ----
                        sps = psp.tile([128, 2, INNER], F32, tag="p4",
                                       name=f"sps{c}")
                        for h in range(H):
                            kt = kT2 if h % 2 == 0 else kT2h
                            qt = qT if h % 2 == 0 else qTh
                            for jg in range(2):
                                jsl = slice(sub * TK + jg * 128,
                                            sub * TK + (jg + 1) * 128)
                                nc.tensor.matmul(
                                    sps[:, jg, h * 64:(h + 1) * 64],
                                    kt[0:64, h // 2, jsl],
                                    qt[0:64, h // 2,
                                       cc * CS:(cc + 1) * CS],
                                    start=True, stop=True)
                        expT = wkp.tile([128, 2, INNER], BF16, tag="expT")
                        nc.scalar.activation(expT, sps, EXP)
                        if stage <= 2 or stage == 17:
                            if sub == 1:
                                continue
                            dbg = wkp.tile([128, DIM], F32, tag="out_sb")
                            nc.vector.tensor_copy(dbg[:, 0:512], expT[:, 0, :])
                            nc.vector.tensor_copy(dbg[:, 512:], v_aug[
                                :, :, :, :].rearrange("p a h w -> p (a h w)")
                                [:, 0:512])
                            nc.sync.dma_start(
                                out=out[pr * 2 * CS:(pr + 1) * 2 * CS, :],
                                in_=dbg)
                            continue

                        # ---- o matmuls into pair psum halves ----
                        # one start per 2KB psum bank region (heads 0-3 / 4-7)
                        for h in range(H):
                            dst = ops_[sub * 64:(sub + 1) * 64, h, 0:65]
                            for jg in range(2):
                                nc.tensor.matmul(
                                    dst, expT[:, jg, h * 64:(h + 1) * 64],
                                    v_aug[:, jg, h, :],
                                    start=(h % 4 == 0 and jg == 0),
                                    stop=False, skip_group_check=True)
                        for h in range(H):
                            nc.tensor.matmul(
                                ops_[sub * 64:(sub + 1) * 64, h, 0:65],
                                expn[:, h // 2, cc * CS:(cc + 1) * CS],
                                nullv_sb[:, h // 2, h % 2, :],
                                start=False, stop=(h % 4 == 3),
                                skip_group_check=True)

                    if stage <= 2 or stage in (15, 17):
                        continue
                    # ---- normalize pair on DVE ----
                    rcol = wkp.tile([128, 8], F32, tag="rcol")
                    nc.vector.reciprocal(rcol, ops_[:, :, 64])
                    o_pair = wkp.tile([128, 8, 64], BF16, tag="o_pair")
                    nc.vector.tensor_mul(
                        o_pair, ops_[:, :, 0:64],
                        rcol.unsqueeze(2).broadcast_to((128, 8, 64)))

                    if stage <= 3:
                        dbg = wkp.tile([128, DIM], F32, tag="out_sb")
                        nc.vector.tensor_copy(dbg[:, 0:512], o_pair[
                            :, :, :].rearrange("p h w -> p (h w)"))
                        nc.vector.memset(dbg[:, 512:], 0.0)
                        nc.sync.dma_start(
                            out=out[pr * 2 * CS:(pr + 1) * 2 * CS, :], in_=dbg)
                        continue
                    # ---- transpose -> oT bf16, out projection ----
                    otr = psp.tile([128, 4, 128], BF16, tag="p2",
                                   name=f"otr{pr}")
                    for et in range(4):
                        nc.tensor.transpose(
                            otr[:, et, :], o_pair[:, 2 * et:2 * et + 2, :],
                            ident)
                    oT = wkp.tile([128, 4, 128], BF16, tag="oT")
                    nc.scalar.copy(oT, otr)
                    outps = psp.tile([128, DIM], F32, tag="p4",
                                     name=f"outps{pr}")
                    for co in range(2):
                        for et in range(4):
                            nc.tensor.matmul(
                                outps[:, co * 512:(co + 1) * 512],
                                oT[:, et, :],
                                wo_sb[:, et, co * 512:(co + 1) * 512],
                                start=(et == 0), stop=(et == 3))
                    out_sb = wkp.tile([128, DIM], F32, tag="out_sb")
                    nc.vector.tensor_add(out_sb, outps, bo_sb)
                    nc.sync.dma_start(
                        out=out[pr * 2 * CS:(pr + 1) * 2 * CS, :], in_=out_sb)

    nc.compile()
    return nc


_CACHED_NC = None


def _get_nc():
    global _CACHED_NC
    if _CACHED_NC is None:
        _CACHED_NC = _build_bass()
    return _CACHED_NC


def _prep_shared(Wq, Wk, Wv, Wo, bo, null_k, null_v, q_pos_emb, k_pos_emb):
    wq_h = np.ascontiguousarray(
        (Wq * SCALE).reshape(8, 128, INNER).transpose(1, 0, 2))

    wk_s = (Wk * (2.0 ** SC_WK)).astype(NPF8)
    wk_r = np.ascontiguousarray(
        wk_s.reshape(8, 128, INNER).transpose(1, 0, 2))
    wk_h = np.ascontiguousarray(
        np.broadcast_to(wk_r[:, :, None, :], (128, 8, 2, INNER)))

    wv32 = Wv * (2.0 ** SC_WV)
    wv_hi = wv32.astype(NPF8)
    wv_lo = (wv32 - wv_hi.astype(np.float32)).astype(NPF8)
    wv_h = np.empty((128, 8, 2, 2, INNER), dtype=NPF8)
    for wi, wmat in enumerate((wv_hi, wv_lo)):
        wr = wmat.reshape(8, 128, INNER).transpose(1, 0, 2)
        wv_h[:, :, wi, 0, :] = wr
        wv_h[:, :, wi, 1, :] = wr

    wo_h = np.ascontiguousarray(
        Wo.reshape(4, 128, DIM).transpose(1, 0, 2)).astype(NPBF)

    # rope-k tables [128 = 2x64 dh halves, 128 pos], psum 2^-12 folded in
    kpe = k_pos_emb[0, 0]                       # [128 pos, 64 dh]
    cos64 = (np.cos(kpe.T) * PSUM_SC).astype(np.float32)   # [64 dh, 128 pos]
    sin64 = (np.sin(kpe.T) * PSUM_SC).astype(np.float32)
    cosk_h = np.concatenate([cos64, cos64], axis=0).astype(NPBF)
    # sin table pre-shifted+signed: reading at src partition yields the value
    # for the dst partition. dst 0:32 <- src 32:64 with -sin[dst]; dst 32:64
    # <- src 0:32 with +sin[dst].
    sin2 = np.empty((64, 128), np.float32)
    sin2[32:64] = -sin64[0:32]
    sin2[0:32] = sin64[32:64]
    sink2_h = np.concatenate([sin2, sin2], axis=0).astype(NPBF)

    qpe63 = q_pos_emb[0, 0, CP]                 # [64]
    cos_q0 = np.cos(qpe63)[:, None].astype(np.float32)
    sgn = np.where(np.arange(64) < 32, -1.0, 1.0)
    sin_q0 = (np.sin(qpe63) * sgn)[:, None].astype(np.float32)
    sp = np.empty_like(sin_q0)
    sp[0:32] = sin_q0[32:64]
    sp[32:64] = sin_q0[0:32]
    cos_q0 = np.concatenate([cos_q0, cos_q0], axis=0)      # [128, 1]
    sin_q0s = np.concatenate([sp, sp], axis=0)

    nk = null_k.reshape(8, 64)                  # [h, dh]
    nullk_h = np.zeros((128, 4, 2), np.float32)
    for h in range(8):
        hb = (h % 2) * 64
        nullk_h[hb:hb + 64, h // 2, h % 2] = nk[h]
    nv = null_v.reshape(8, 64)
    nullv_h = np.zeros((2, 4, 2, 65), np.float32)
    for h in range(8):
        nullv_h[h % 2, h // 2, h % 2, 0:64] = nv[h]
        nullv_h[h % 2, h // 2, h % 2, 64] = 1.0

    return {
        "wq": wq_h, "wk8": wk_h, "wv8": wv_h, "wo": wo_h, "bo": bo,
        "cosk": cosk_h, "sink2": sink2_h,
        "cos_q0": cos_q0, "sin_q0s": sin_q0s,
        "nullk2": nullk_h.astype(NPBF), "nullv2": nullv_h.astype(NPBF),
    }


def kernel(x, context, q_pos_emb, k_pos_emb, Wq, Wk, Wv, Wo, bo, null_k, null_v):
    x = np.asarray(x, dtype=np.float32)
    context = np.asarray(context, dtype=np.float32)
    q_pos_emb = np.asarray(q_pos_emb, dtype=np.float32)
    k_pos_emb = np.asarray(k_pos_emb, dtype=np.float32)
    Wq = np.asarray(Wq, dtype=np.float32)
    Wk = np.asarray(Wk, dtype=np.float32)
    Wv = np.asarray(Wv, dtype=np.float32)
    Wo = np.asarray(Wo, dtype=np.float32)
    bo = np.asarray(bo, dtype=np.float32)
    null_k = np.asarray(null_k, dtype=np.float32)
    null_v = np.asarray(null_v, dtype=np.float32)

    xs = np.zeros_like(x)
    xs[:, : N - CP] = x[:, CP:]
    xc = xs.reshape(BK, CS, DIM)
    ctx = context.reshape(BK, TK, DIM)

    shared = _prep_shared(Wq, Wk, Wv, Wo, bo, null_k, null_v,
                          q_pos_emb, k_pos_emb)

    in_maps = []
    for c in range(N_CORES):
        sl = slice(c * CPC, (c + 1) * CPC)
        xT_c = np.ascontiguousarray(xc[sl].reshape(TQ, DIM).T)
        ctxT_c = np.ascontiguousarray(
            ctx[sl].reshape(TCTX, DIM).T) * (2.0 ** SC_CTX)
        hi = ctxT_c.astype(NPF8)
        lo = (ctxT_c - hi.astype(np.float32)).astype(NPF8)
        ctx8_c = np.stack([hi, lo], axis=0)     # [2, DIM, TCTX]
        in_maps.append({"xT": xT_c, "ctx8": ctx8_c, **shared})

    nc = _get_nc()
    res = run_bass_kernel_spmd(nc, in_maps, core_ids=list(range(N_CORES)))

    out_full = np.concatenate([res.results[c]["out"] for c in range(N_CORES)],
                              axis=0)                      # [BK*CS, DIM]
    o = out_full.reshape(B, K_CHUNKS * CS, DIM)
    final = np.concatenate(
        [np.zeros((B, CP, DIM), np.float32), o[:, : K_CHUNKS * CS - CP]],
        axis=1)
    return final


# revision 22
# speedup vs baseline: 1.4071x; 1.0004x over previous
"""Trainium2 Bass kernel for nn_ChunkedCrossAttention_85907935855128.

Self-contained: hardcodes shapes/sharding. Accepts FULL inputs, returns FULL
output. Shards the fused (b*k_chunks) chunk axis across 8 NeuronCores.

v2 dataflow per core (32 chunks):
  - k-projection: fp8e4m3 DoubleRow, activation hi+lo error compensation
    (ctx quantized on host to hi+residual fp8 pair; Wk single fp8).
  - v-projection: fully-compensated fp8 DoubleRow ((hi+lo)@(Whi+Wlo)); same
    cost as bf16 but lets ctx live in SBUF as the fp8 pair only.
  - q-projection: fp32r. out-projection: bf16.
  - rope-k on DVE (partition-shifted sin muls, psum scale folded in tables),
    no PE perm matmul.
  - attention: pair-stacked o psum (2 chunks on partition halves), exp bf16
    on ACT, softmax sum via ones column, null-k/v via zero-masked K=2 mms.
"""
import os
# bass2jax executes via the axon PJRT platform; a CPU pin would hide the cores.
if os.environ.get("JAX_PLATFORMS", "") in ("cpu",):
    del os.environ["JAX_PLATFORMS"]

import numpy as np
import ml_dtypes

import concourse.bacc as bacc
import concourse.bass as bass
import concourse.mybir as mybir
import concourse.tile as tile
from concourse.bass_utils import run_bass_kernel_spmd
from concourse.masks import make_identity

F32 = mybir.dt.float32
F32R = mybir.dt.float32r
BF16 = mybir.dt.bfloat16
FP8 = mybir.dt.float8e4
NPF8 = ml_dtypes.float8_e4m3
NPBF = ml_dtypes.bfloat16
DR = mybir.MatmulPerfMode.DoubleRow
EXP = mybir.ActivationFunctionType.Exp
COPY = mybir.ActivationFunctionType.Copy

CS, CP, H, DH = 64, 63, 8, 64
SCALE = DH ** -0.5
N_CORES = 8
B, N, DIM = 4, 4096, 1024
K_CHUNKS, R, RLEN = 64, 2, 128
TK = R * RLEN                 # 256 ctx tokens / chunk
BK = B * K_CHUNKS             # 256 chunks
CPC = BK // N_CORES           # 32 chunks / core
TQ = CPC * CS                 # 2048 q tokens / core
TCTX = CPC * TK               # 8192 ctx tokens / core
INNER = H * DH                # 512
QG = 4                        # chunks per q-projection group
NQG = CPC // QG               # 8 q groups / core

SC_CTX = 3                    # ctx * 2^3 before fp8
SC_WK = 9                     # Wk * 2^9
SC_WV = 9
PSUM_SC = 2.0 ** -(SC_CTX + SC_WK)   # folded into rope tables / v evac


def _build_bass(num_devices=N_CORES, stage=4):
    nc = bacc.Bacc("TRN2", target_bir_lowering=False, debug=False,
                   num_devices=num_devices)

    xT = nc.dram_tensor("xT", (DIM, TQ), F32, kind="ExternalInput")
    ctx8 = nc.dram_tensor("ctx8", (2, DIM, TCTX), FP8, kind="ExternalInput")
    wq = nc.dram_tensor("wq", (128, 8, INNER), F32, kind="ExternalInput")
    wk8 = nc.dram_tensor("wk8", (128, 8, 2, INNER), FP8, kind="ExternalInput")
    wv8 = nc.dram_tensor("wv8", (128, 8, 2, 2, INNER), FP8, kind="ExternalInput")
    wo = nc.dram_tensor("wo", (128, 4, DIM), BF16, kind="ExternalInput")
    bo = nc.dram_tensor("bo", (DIM,), F32, kind="ExternalInput")
    cosk = nc.dram_tensor("cosk", (128, 128), BF16, kind="ExternalInput")
    sink2 = nc.dram_tensor("sink2", (128, 128), BF16, kind="ExternalInput")
    cos_q0 = nc.dram_tensor("cos_q0", (128, 1), F32, kind="ExternalInput")
    sin_q0s = nc.dram_tensor("sin_q0s", (128, 1), F32, kind="ExternalInput")
    nullk2 = nc.dram_tensor("nullk2", (128, 4, 2), BF16, kind="ExternalInput")
    nullv2 = nc.dram_tensor("nullv2", (2, 4, 2, 65), BF16, kind="ExternalInput")
    out = nc.dram_tensor("out", (TQ, DIM), F32, kind="ExternalOutput")

    with tile.TileContext(nc) as tc:
        with tc.tile_pool(name="consts", bufs=1) as cp_, \
             tc.tile_pool(name="wk", bufs=2) as wkp, \
             tc.tile_pool(name="ps", bufs=2, space="PSUM") as psp:

            # ---- constants (ordered by first use) ----
            wq_sb = cp_.tile([128, 8, INNER], F32R)
            nc.sync.dma_start(out=wq_sb, in_=wq[:, :, :].bitcast(F32R))
            wk_sb = cp_.tile([128, 8, 2, INNER], FP8)
            nc.sync.dma_start(out=wk_sb, in_=wk8[:, :, :, :])
            cosk_sb = cp_.tile([128, 128], BF16)
            nc.sync.dma_start(out=cosk_sb, in_=cosk[:, :])
            sink2_sb = cp_.tile([128, 128], BF16)
            nc.sync.dma_start(out=sink2_sb, in_=sink2[:, :])
            cosq_sb = cp_.tile([128, 1], F32)
            nc.sync.dma_start(out=cosq_sb, in_=cos_q0[:, :])
            sinq_sb = cp_.tile([128, 1], F32)
            nc.sync.dma_start(out=sinq_sb, in_=sin_q0s[:, :])
            nullk_sb = cp_.tile([128, 4, 2], BF16)
            nc.sync.dma_start(out=nullk_sb, in_=nullk2[:, :, :])
            nullv_sb = cp_.tile([2, 4, 2, 65], BF16)
            nc.sync.dma_start(out=nullv_sb, in_=nullv2[:, :, :, :])
            ident = cp_.tile([128, 128], BF16)
            make_identity(nc, ident)
            wv_sb = cp_.tile([128, 8, 2, 2, INNER], FP8)
            nc.sync.dma_start(out=wv_sb, in_=wv8[:, :, :, :, :])
            wo_sb = cp_.tile([128, 4, DIM], BF16)
            nc.sync.dma_start(out=wo_sb, in_=wo[:, :, :])
            bo_sb = cp_.tile([128, DIM], F32)
            nc.sync.dma_start(out=bo_sb, in_=bass.AP(
                tensor=bo, offset=0, ap=[[0, 128], [1, DIM]]))

            def head(g):
                """q-projection + rope-q + null-k sims for one group."""
                gcols = slice(g * QG * CS, (g + 1) * QG * CS)
                # ---- q projection (fp32r): qps [128=2 heads, 2, 256] x2 ----
                xT_sb = wkp.tile([128, 8, QG * CS], F32R, tag="xT")
                nc.sync.dma_start(out=xT_sb, in_=xT[:, :].rearrange(
                    "(dt p) t -> p dt t", p=128)[:, :, gcols].bitcast(F32R))
                qps = [psp.tile([128, 2, QG * CS], F32, tag="p2",
                                name=f"qps{g}_{i}") for i in range(2)]
                for it in range(4):
                    for dt in range(8):
                        nc.tensor.matmul(
                            qps[it // 2][:, it % 2, :],
                            wq_sb[:, dt, it * 128:(it + 1) * 128],
                            xT_sb[:, dt, :],
                            start=(dt == 0), stop=(dt == 7))
                # qT [128 = head-pair dh, 4 hp, 256] bf16
                qT = wkp.tile([128, 4, QG * CS], BF16, tag="qT")
                for it in range(4):
                    nc.scalar.copy(qT[:, it, :], qps[it // 2][:, it % 2, :])
                # rope-q: fix token 0 of each chunk (cols ::CS), both halves
                qcols = qT[:, :, :].rearrange(
                    "p h (c w) -> p h c w", w=CS)[:, :, :, 0]   # [128, 4, QG]
                t1q = wkp.tile([128, 4, QG], BF16, tag="t1q")
                nc.vector.tensor_mul(
                    t1q, qcols, cosq_sb.unsqueeze(2).broadcast_to((128, 4, QG)))
                t2q = wkp.tile([128, 4, QG], BF16, tag="t2q")
                for (dst, src) in ((0, 32), (32, 0), (64, 96), (96, 64)):
                    nc.vector.tensor_mul(
                        t2q[dst:dst + 32, :, :],
                        qT[:, :, :].rearrange(
                            "p h (c w) -> p h c w", w=CS)[src:src + 32, :, :, 0],
                        sinq_sb[src:src + 32, :].unsqueeze(2)
                        .broadcast_to((32, 4, QG)))
                nc.vector.tensor_add(qcols, t1q, t2q)
                # odd heads live on partitions 64-127; PE operands must sit at
                # base 0 on hw, so keep a base-0 copy for the sim matmuls
                qTh = wkp.tile([64, 4, QG * CS], BF16, tag="qTh")
                nc.vector.tensor_copy(qTh, qT[64:128, :, :])

                # ---- null-k sims -> expn bf16 [2, 4 hp, 256] ----
                expn = wkp.tile([2, 4, QG * CS], BF16, tag="expn")
                for half in range(2):
                    nps = psp.tile([2, 2, QG * CS], F32, tag="p2",
                                   name=f"nps{g}_{half}")
                    for hh in range(2):
                        hp = half * 2 + hh
                        nc.tensor.matmul(
                            nps[:, hh, :], nullk_sb[:, hp, :], qT[:, hp, :],
                            start=True, stop=True)
                    nc.scalar.activation(
                        expn[:, half * 2:(half + 1) * 2, :], nps, EXP)
                return qT, qTh, expn

            cur = head(0)
            for g in range(NQG):          # 8 groups of 4 chunks
                qT, qTh, expn = cur
                for pp in range(2):       # pairs within the group
                    pr = g * 2 + pp       # global pair index
                    pcols = slice(pr * 2 * TK, (pr + 1) * 2 * TK)
                    # ---- ctx fp8 pair [128, 8 dt, 2 hi/lo, 512 tok] ----
                    ctx_sb = wkp.tile([128, 8, 2, 2 * TK], FP8, tag="ctx")
                    for s in range(2):
                        nc.sync.dma_start(
                            out=ctx_sb[:, :, s, :],
                            in_=ctx8[s, :, :].rearrange(
                                "(dt p) t -> p dt t", p=128)[:, :, pcols])

                    # ---- k projection (fp8 DR, act-compensated) ----
                    # kraw [128 = head-pair dh, 4 hp, 512] bf16
                    kraw = wkp.tile([128, 4, 2 * TK], BF16, tag="kraw")
                    for it in range(4):
                        kps = psp.tile([128, 2 * TK], F32, tag="p2",
                                       name=f"kps{pr}_{it}")
                        for dt in range(8):
                            nc.tensor.matmul(
                                kps, wk_sb[:, dt, :, it * 128:(it + 1) * 128],
                                ctx_sb[:, dt, :, :],
                                start=(dt == 0), stop=(dt == 7),
                                perf_mode=DR)
                        nc.scalar.copy(kraw[:, it, :], kps)
                    # ---- rope-k on DVE: kT2 = cos*k + sin_shift*k_shift ----
                    t1k = wkp.tile([128, 4, 2 * TK], BF16, tag="t1k")
                    nc.vector.tensor_mul(
                        t1k[:, :, :].rearrange("p h (r c) -> p h r c", r=4),
                        kraw[:, :, :].rearrange("p h (r c) -> p h r c", r=4),
                        cosk_sb.unsqueeze(1).unsqueeze(2)
                        .broadcast_to((128, 4, 4, 128)))
                    t2k = wkp.tile([128, 4, 2 * TK], BF16, tag="t2k")
                    for (dst, src) in ((0, 32), (32, 0), (64, 96), (96, 64)):
                        nc.vector.tensor_mul(
                            t2k[dst:dst + 32, :, :].rearrange(
                                "p h (r c) -> p h r c", r=4),
                            kraw[src:src + 32, :, :].rearrange(
                                "p h (r c) -> p h r c", r=4),
                            sink2_sb[src:src + 32, :].unsqueeze(1).unsqueeze(2)
                            .broadcast_to((32, 4, 4, 128)))
                    kT2 = wkp.tile([128, 4, 2 * TK], BF16, tag="kT2")
                    nc.vector.tensor_add(kT2, t1k, t2k)
                    kT2h = wkp.tile([64, 4, 2 * TK], BF16, tag="kT2h")
                    nc.vector.tensor_copy(kT2h, kT2[64:128, :, :])

                    if stage <= 1:
                        dbg = wkp.tile([128, DIM], F32, tag="out_sb")
                        nc.vector.tensor_copy(dbg[:, 0:512], kT2[:, 0, :])
                        nc.vector.memset(dbg[:, 512:], 0.0)
                        nc.sync.dma_start(
                            out=out[pr * 2 * CS:(pr + 1) * 2 * CS, :], in_=dbg)
                        continue

                    # ---- per-pair o psum [128 = 2 chunks, 8 h, 128] ----
                    ops_ = None
                    if stage in (3, 4):
                        ops_ = psp.tile([128, 8, 128], F32, tag="p4",
                                        name=f"ops{pr}")
                    for sub in range(2):
                        c = pr * 2 + sub            # global chunk
                        cc = pp * 2 + sub           # chunk within group
                        # ---- v projection (fp8 DR, fully compensated) ----
                        v_aug = wkp.tile([128, 2, 8, 65], BF16, tag="v_aug")
                        if stage == 17:      # skip v-proj, dummy v_aug
                            nc.vector.memset(v_aug[:, :, :, :], 1.0)
                        for tg in range(2 if stage != 17 else 0):
                            vps = psp.tile([128, INNER], F32, tag="pv",
                                           name=f"vps{c}_{tg}")
                            tsl = slice(sub * TK + tg * 128,
                                        sub * TK + (tg + 1) * 128)
                            for dt in range(8):
                                for w in range(2):
                                    nc.tensor.matmul(
                                        vps,
                                        ctx_sb[:, dt, :, tsl],
                                        wv_sb[:, dt, w, :, :],
                                        start=(dt == 0 and w == 0),
                                        stop=(dt == 7 and w == 1),
                                        perf_mode=DR)
                            nc.scalar.activation(
                                v_aug[:, tg, :, 0:64],
                                vps[:, :].rearrange("p (h w) -> p h w", h=8),
                                COPY, scale=PSUM_SC)
                        nc.gpsimd.memset(v_aug[:, :, :, 64:65], 1.0)
                        if stage == 15:      # v-proj only
                            if sub == 1:
                                continue
                            dbg = wkp.tile([128, DIM], F32, tag="out_sb")
                            nc.vector.tensor_copy(
                                dbg[:, 0:512].rearrange(
                                    "p (h w) -> p h w", h=8),
                                v_aug[:, 0, :, 0:64])
                            nc.vector.memset(dbg[:, 512:], 0.0)
                            nc.sync.dma_start(
                                out=out[pr * 2 * CS:(pr + 1) * 2 * CS, :],
                                in_=dbg)
                            continue

                        # ---- sim [128 j, 2 jg, (h, i)] ----
                        sps = psp.tile([128, 2, INNER], F32, tag="p4",
                                       name=f"sps{c}")
                        for h in range(H):
                            kt = kT2 if h % 2 == 0 else kT2h
                            qt = qT if h % 2 == 0 else qTh
                            for jg in range(2):
                                jsl = slice(sub * TK + jg * 128,
                                            sub * TK + (jg + 1) * 128)
                                nc.tensor.matmul(
                                    sps[:, jg, h * 64:(h + 1) * 64],
                                    kt[0:64, h // 2, jsl],
                                    qt[0:64, h // 2,
                                       cc * CS:(cc + 1) * CS],
                                    start=True, stop=True)
                        expT = wkp.tile([128, 2, INNER], BF16, tag="expT")
                        nc.scalar.activation(expT, sps, EXP)
                        if stage <= 2 or stage == 17:
                            if sub == 1:
                                continue
                            dbg = wkp.tile([128, DIM], F32, tag="out_sb")
                            nc.vector.tensor_copy(dbg[:, 0:512], expT[:, 0, :])
                            nc.vector.tensor_copy(dbg[:, 512:], v_aug[
                                :, :, :, :].rearrange("p a h w -> p (a h w)")
                                [:, 0:512])
                            nc.sync.dma_start(
                                out=out[pr * 2 * CS:(pr + 1) * 2 * CS, :],
                                in_=dbg)
                            continue

                        # ---- o matmuls into pair psum halves ----
                        # one start per 2KB psum bank region (heads 0-3 / 4-7)
                        for h in range(H):
                            dst = ops_[sub * 64:(sub + 1) * 64, h, 0:65]
                            for jg in range(2):
                                nc.tensor.matmul(
                                    dst, expT[:, jg, h * 64:(h + 1) * 64],
                                    v_aug[:, jg, h, :],
                                    start=(h % 4 == 0 and jg == 0),
                                    stop=False, skip_group_check=True)
                        for h in range(H):
                            nc.tensor.matmul(
                                ops_[sub * 64:(sub + 1) * 64, h, 0:65],
                                expn[:, h // 2, cc * CS:(cc + 1) * CS],
                                nullv_sb[:, h // 2, h % 2, :],
                                start=False, stop=(h % 4 == 3),
                                skip_group_check=True)

                    if stage <= 2 or stage in (15, 17):
                        continue
                    # ---- normalize pair on DVE ----
                    rcol = wkp.tile([128, 8], F32, tag="rcol")
                    nc.vector.reciprocal(rcol, ops_[:, :, 64])
                    o_pair = wkp.tile([128, 8, 64], BF16, tag="o_pair")
                    nc.vector.tensor_mul(
                        o_pair, ops_[:, :, 0:64],
                        rcol.unsqueeze(2).broadcast_to((128, 8, 64)))

                    if stage <= 3:
                        dbg = wkp.tile([128, DIM], F32, tag="out_sb")
                        nc.vector.tensor_copy(dbg[:, 0:512], o_pair[
                            :, :, :].rearrange("p h w -> p (h w)"))
                        nc.vector.memset(dbg[:, 512:], 0.0)
                        nc.sync.dma_start(
                            out=out[pr * 2 * CS:(pr + 1) * 2 * CS, :], in_=dbg)
                        continue
                    # ---- transpose -> oT bf16, out projection ----
                    otr = psp.tile([128, 4, 128], BF16, tag="p2",
                                   name=f"otr{pr}")
                    for et in range(4):
                        nc.tensor.transpose(
                            otr[:, et, :], o_pair[:, 2 * et:2 * et + 2, :],
                            ident)
                    oT = wkp.tile([128, 4, 128], BF16, tag="oT")
                    nc.scalar.copy(oT, otr)
                    outps = psp.tile([128, DIM], F32, tag="p4",
                                     name=f"outps{pr}")
                    for co in range(2):
                        for et in range(4):
                            nc.tensor.matmul(
                                outps[:, co * 512:(co + 1) * 512],
                                oT[:, et, :],
                                wo_sb[:, et, co * 512:(co + 1) * 512],
                                start=(et == 0), stop=(et == 3))
                    out_sb = wkp.tile([128, DIM], F32, tag="out_sb")
                    nc.vector.tensor_add(out_sb, outps, bo_sb)
                    nc.sync.dma_start(
                        out=out[pr * 2 * CS:(pr + 1) * 2 * CS, :], in_=out_sb)
                    if pp == 0 and g + 1 < NQG:
                        cur = head(g + 1)

    nc.compile()
    return nc


_CACHED_NC = None


def _get_nc():
    global _CACHED_NC
    if _CACHED_NC is None:
        _CACHED_NC = _build_bass()
    return _CACHED_NC


def _prep_shared(Wq, Wk, Wv, Wo, bo, null_k, null_v, q_pos_emb, k_pos_emb):
    wq_h = np.ascontiguousarray(
        (Wq * SCALE).reshape(8, 128, INNER).transpose(1, 0, 2))

    wk_s = (Wk * (2.0 ** SC_WK)).astype(NPF8)
    wk_r = np.ascontiguousarray(
        wk_s.reshape(8, 128, INNER).transpose(1, 0, 2))
    wk_h = np.ascontiguousarray(
        np.broadcast_to(wk_r[:, :, None, :], (128, 8, 2, INNER)))

    wv32 = Wv * (2.0 ** SC_WV)
    wv_hi = wv32.astype(NPF8)
    wv_lo = (wv32 - wv_hi.astype(np.float32)).astype(NPF8)
    wv_h = np.empty((128, 8, 2, 2, INNER), dtype=NPF8)
    for wi, wmat in enumerate((wv_hi, wv_lo)):
        wr = wmat.reshape(8, 128, INNER).transpose(1, 0, 2)
        wv_h[:, :, wi, 0, :] = wr
        wv_h[:, :, wi, 1, :] = wr

    wo_h = np.ascontiguousarray(
        Wo.reshape(4, 128, DIM).transpose(1, 0, 2)).astype(NPBF)

    # rope-k tables [128 = 2x64 dh halves, 128 pos], psum 2^-12 folded in
    kpe = k_pos_emb[0, 0]                       # [128 pos, 64 dh]
    cos64 = (np.cos(kpe.T) * PSUM_SC).astype(np.float32)   # [64 dh, 128 pos]
    sin64 = (np.sin(kpe.T) * PSUM_SC).astype(np.float32)
    cosk_h = np.concatenate([cos64, cos64], axis=0).astype(NPBF)
    # sin table pre-shifted+signed: reading at src partition yields the value
    # for the dst partition. dst 0:32 <- src 32:64 with -sin[dst]; dst 32:64
    # <- src 0:32 with +sin[dst].
    sin2 = np.empty((64, 128), np.float32)
    sin2[32:64] = -sin64[0:32]
    sin2[0:32] = sin64[32:64]
    sink2_h = np.concatenate([sin2, sin2], axis=0).astype(NPBF)

    qpe63 = q_pos_emb[0, 0, CP]                 # [64]
    cos_q0 = np.cos(qpe63)[:, None].astype(np.float32)
    sgn = np.where(np.arange(64) < 32, -1.0, 1.0)
    sin_q0 = (np.sin(qpe63) * sgn)[:, None].astype(np.float32)
    sp = np.empty_like(sin_q0)
    sp[0:32] = sin_q0[32:64]
    sp[32:64] = sin_q0[0:32]
    cos_q0 = np.concatenate([cos_q0, cos_q0], axis=0)      # [128, 1]
    sin_q0s = np.concatenate([sp, sp], axis=0)

    nk = null_k.reshape(8, 64)                  # [h, dh]
    nullk_h = np.zeros((128, 4, 2), np.float32)
    for h in range(8):
        hb = (h % 2) * 64
        nullk_h[hb:hb + 64, h // 2, h % 2] = nk[h]
    nv = null_v.reshape(8, 64)
    nullv_h = np.zeros((2, 4, 2, 65), np.float32)
    for h in range(8):
        nullv_h[h % 2, h // 2, h % 2, 0:64] = nv[h]
        nullv_h[h % 2, h // 2, h % 2, 64] = 1.0

    return {
        "wq": wq_h, "wk8": wk_h, "wv8": wv_h, "wo": wo_h, "bo": bo,
        "cosk": cosk_h, "sink2": sink2_h,
        "cos_q0": cos_q0, "sin_q0s": sin_q0s,
        "nullk2": nullk_h.astype(NPBF), "nullv2": nullv_h.astype(NPBF),
    }


def kernel(x, context, q_pos_emb, k_pos_emb, Wq, Wk, Wv, Wo, bo, null_k, null_v):
    x = np.asarray(x, dtype=np.float32)
    context = np.asarray(context, dtype=np.float32)
    q_pos_emb = np.asarray(q_pos_emb, dtype=np.float32)
    k_pos_emb = np.asarray(k_pos_emb, dtype=np.float32)
    Wq = np.asarray(Wq, dtype=np.float32)
    Wk = np.asarray(Wk, dtype=np.float32)
    Wv = np.asarray(Wv, dtype=np.float32)
    Wo = np.asarray(Wo, dtype=np.float32)
    bo = np.asarray(bo, dtype=np.float32)
    null_k = np.asarray(null_k, dtype=np.float32)
    null_v = np.asarray(null_v, dtype=np.float32)

    xs = np.zeros_like(x)
    xs[:, : N - CP] = x[:, CP:]
    xc = xs.reshape(BK, CS, DIM)
    ctx = context.reshape(BK, TK, DIM)

    shared = _prep_shared(Wq, Wk, Wv, Wo, bo, null_k, null_v,
                          q_pos_emb, k_pos_emb)

    in_maps = []
    for c in range(N_CORES):
        sl = slice(c * CPC, (c + 1) * CPC)
        xT_c = np.ascontiguousarray(xc[sl].reshape(TQ, DIM).T)
        ctxT_c = np.ascontiguousarray(
            ctx[sl].reshape(TCTX, DIM).T) * (2.0 ** SC_CTX)
        hi = ctxT_c.astype(NPF8)
        lo = (ctxT_c - hi.astype(np.float32)).astype(NPF8)
        ctx8_c = np.stack([hi, lo], axis=0)     # [2, DIM, TCTX]
        in_maps.append({"xT": xT_c, "ctx8": ctx8_c, **shared})

    nc = _get_nc()
    res = run_bass_kernel_spmd(nc, in_maps, core_ids=list(range(N_CORES)))

    out_full = np.concatenate([res.results[c]["out"] for c in range(N_CORES)],
                              axis=0)                      # [BK*CS, DIM]
    o = out_full.reshape(B, K_CHUNKS * CS, DIM)
    final = np.concatenate(
        [np.zeros((B, CP, DIM), np.float32), o[:, : K_CHUNKS * CS - CP]],
        axis=1)
    return final


# revision 24
# speedup vs baseline: 1.4075x; 1.0003x over previous
"""Trainium2 Bass kernel for nn_ChunkedCrossAttention_85907935855128.

Self-contained: hardcodes shapes/sharding. Accepts FULL inputs, returns FULL
output. Shards the fused (b*k_chunks) chunk axis across 8 NeuronCores.

v2 dataflow per core (32 chunks):
  - k-projection: fp8e4m3 DoubleRow, activation hi+lo error compensation
    (ctx quantized on host to hi+residual fp8 pair; Wk single fp8).
  - v-projection: fully-compensated fp8 DoubleRow ((hi+lo)@(Whi+Wlo)); same
    cost as bf16 but lets ctx live in SBUF as the fp8 pair only.
  - q-projection: fp32r. out-projection: bf16.
  - rope-k on DVE (partition-shifted sin muls, psum scale folded in tables),
    no PE perm matmul.
  - attention: pair-stacked o psum (2 chunks on partition halves), exp bf16
    on ACT, softmax sum via ones column, null-k/v via zero-masked K=2 mms.
"""
import os
# bass2jax executes via the axon PJRT platform; a CPU pin would hide the cores.
if os.environ.get("JAX_PLATFORMS", "") in ("cpu",):
    del os.environ["JAX_PLATFORMS"]

import numpy as np
import ml_dtypes

import concourse.bacc as bacc
import concourse.bass as bass
import concourse.mybir as mybir
import concourse.tile as tile
from concourse.bass_utils import run_bass_kernel_spmd
from concourse.masks import make_identity

F32 = mybir.dt.float32
F32R = mybir.dt.float32r
BF16 = mybir.dt.bfloat16
FP8 = mybir.dt.float8e4
NPF8 = ml_dtypes.float8_e4m3
NPBF = ml_dtypes.bfloat16
DR = mybir.MatmulPerfMode.DoubleRow
EXP = mybir.ActivationFunctionType.Exp
COPY = mybir.ActivationFunctionType.Copy

CS, CP, H, DH = 64, 63, 8, 64
SCALE = DH ** -0.5
N_CORES = 8
B, N, DIM = 4, 4096, 1024
K_CHUNKS, R, RLEN = 64, 2, 128
TK = R * RLEN                 # 256 ctx tokens / chunk
BK = B * K_CHUNKS             # 256 chunks
CPC = BK // N_CORES           # 32 chunks / core
TQ = CPC * CS                 # 2048 q tokens / core
TCTX = CPC * TK               # 8192 ctx tokens / core
INNER = H * DH                # 512
QG = 4                        # chunks per q-projection group
NQG = CPC // QG               # 8 q groups / core

SC_CTX = 3                    # ctx * 2^3 before fp8
SC_WK = 9                     # Wk * 2^9
SC_WV = 9
PSUM_SC = 2.0 ** -(SC_CTX + SC_WK)   # folded into rope tables / v evac


def _build_bass(num_devices=N_CORES, stage=4):
    nc = bacc.Bacc("TRN2", target_bir_lowering=False, debug=False,
                   num_devices=num_devices)

    xT = nc.dram_tensor("xT", (DIM, TQ), F32, kind="ExternalInput")
    ctx8 = nc.dram_tensor("ctx8", (2, DIM, TCTX), FP8, kind="ExternalInput")
    wq = nc.dram_tensor("wq", (128, 8, INNER), F32, kind="ExternalInput")
    wk8 = nc.dram_tensor("wk8", (128, 8, 2, INNER), FP8, kind="ExternalInput")
    wv8 = nc.dram_tensor("wv8", (128, 8, 2, 2, INNER), FP8, kind="ExternalInput")
    wo = nc.dram_tensor("wo", (128, 4, DIM), BF16, kind="ExternalInput")
    bo = nc.dram_tensor("bo", (DIM,), F32, kind="ExternalInput")
    cosk = nc.dram_tensor("cosk", (128, 128), BF16, kind="ExternalInput")
    sink2 = nc.dram_tensor("sink2", (128, 128), BF16, kind="ExternalInput")
    cos_q0 = nc.dram_tensor("cos_q0", (128, 1), F32, kind="ExternalInput")
    sin_q0s = nc.dram_tensor("sin_q0s", (128, 1), F32, kind="ExternalInput")
    nullk2 = nc.dram_tensor("nullk2", (128, 4, 2), BF16, kind="ExternalInput")
    nullv2 = nc.dram_tensor("nullv2", (2, 4, 2, 65), BF16, kind="ExternalInput")
    out = nc.dram_tensor("out", (TQ, DIM), F32, kind="ExternalOutput")

    with tile.TileContext(nc) as tc:
        with tc.tile_pool(name="consts", bufs=1) as cp_, \
             tc.tile_pool(name="wk", bufs=2) as wkp, \
             tc.tile_pool(name="ps", bufs=2, space="PSUM") as psp:

            # ---- constants (ordered by first use) ----
            wq_sb = cp_.tile([128, 8, INNER], F32R)
            nc.sync.dma_start(out=wq_sb, in_=wq[:, :, :].bitcast(F32R))
            wk_sb = cp_.tile([128, 8, 2, INNER], FP8)
            nc.sync.dma_start(out=wk_sb, in_=wk8[:, :, :, :])
            cosk_sb = cp_.tile([128, 128], BF16)
            nc.sync.dma_start(out=cosk_sb, in_=cosk[:, :])
            sink2_sb = cp_.tile([128, 128], BF16)
            nc.sync.dma_start(out=sink2_sb, in_=sink2[:, :])
            cosq_sb = cp_.tile([128, 1], F32)
            nc.sync.dma_start(out=cosq_sb, in_=cos_q0[:, :])
            sinq_sb = cp_.tile([128, 1], F32)
            nc.sync.dma_start(out=sinq_sb, in_=sin_q0s[:, :])
            nullk_sb = cp_.tile([128, 4, 2], BF16)
            nc.sync.dma_start(out=nullk_sb, in_=nullk2[:, :, :])
            nullv_sb = cp_.tile([2, 4, 2, 65], BF16)
            nc.sync.dma_start(out=nullv_sb, in_=nullv2[:, :, :, :])
            ident = cp_.tile([128, 128], BF16)
            make_identity(nc, ident)
            wv_sb = cp_.tile([128, 8, 2, 2, INNER], FP8)
            nc.sync.dma_start(out=wv_sb, in_=wv8[:, :, :, :, :])
            wo_sb = cp_.tile([128, 4, DIM], BF16)
            nc.sync.dma_start(out=wo_sb, in_=wo[:, :, :])
            bo_sb = cp_.tile([128, DIM], F32)
            nc.sync.dma_start(out=bo_sb, in_=bass.AP(
                tensor=bo, offset=0, ap=[[0, 128], [1, DIM]]))

            def head(g):
                """q-projection + rope-q + null-k sims for one group."""
                gcols = slice(g * QG * CS, (g + 1) * QG * CS)
                # ---- q projection (fp32r): qps [128=2 heads, 2, 256] x2 ----
                xT_sb = wkp.tile([128, 8, QG * CS], F32R, tag="xT")
                nc.sync.dma_start(out=xT_sb, in_=xT[:, :].rearrange(
                    "(dt p) t -> p dt t", p=128)[:, :, gcols].bitcast(F32R))
                qps = [psp.tile([128, 2, QG * CS], F32, tag="p2",
                                name=f"qps{g}_{i}") for i in range(2)]
                for it in range(4):
                    for dt in range(8):
                        nc.tensor.matmul(
                            qps[it // 2][:, it % 2, :],
                            wq_sb[:, dt, it * 128:(it + 1) * 128],
                            xT_sb[:, dt, :],
                            start=(dt == 0), stop=(dt == 7))
                # qT [128 = head-pair dh, 4 hp, 256] bf16
                qT = wkp.tile([128, 4, QG * CS], BF16, tag="qT")
                for it in range(4):
                    nc.scalar.copy(qT[:, it, :], qps[it // 2][:, it % 2, :])
                # rope-q: fix token 0 of each chunk (cols ::CS), both halves
                qcols = qT[:, :, :].rearrange(
                    "p h (c w) -> p h c w", w=CS)[:, :, :, 0]   # [128, 4, QG]
                t1q = wkp.tile([128, 4, QG], BF16, tag="t1q")
                nc.vector.tensor_mul(
                    t1q, qcols, cosq_sb.unsqueeze(2).broadcast_to((128, 4, QG)))
                t2q = wkp.tile([128, 4, QG], BF16, tag="t2q")
                for (dst, src) in ((0, 32), (32, 0), (64, 96), (96, 64)):
                    nc.vector.tensor_mul(
                        t2q[dst:dst + 32, :, :],
                        qT[:, :, :].rearrange(
                            "p h (c w) -> p h c w", w=CS)[src:src + 32, :, :, 0],
                        sinq_sb[src:src + 32, :].unsqueeze(2)
                        .broadcast_to((32, 4, QG)))
                nc.vector.tensor_add(qcols, t1q, t2q)
                # odd heads live on partitions 64-127; PE operands must sit at
                # base 0 on hw, so keep a base-0 copy for the sim matmuls
                qTh = wkp.tile([64, 4, QG * CS], BF16, tag="qTh")
                nc.vector.tensor_copy(qTh, qT[64:128, :, :])

                # ---- null-k sims -> expn bf16 [2, 4 hp, 256] ----
                expn = wkp.tile([2, 4, QG * CS], BF16, tag="expn")
                for half in range(2):
                    nps = psp.tile([2, 2, QG * CS], F32, tag="p2",
                                   name=f"nps{g}_{half}")
                    for hh in range(2):
                        hp = half * 2 + hh
                        nc.tensor.matmul(
                            nps[:, hh, :], nullk_sb[:, hp, :], qT[:, hp, :],
                            start=True, stop=True)
                    nc.scalar.activation(
                        expn[:, half * 2:(half + 1) * 2, :], nps, EXP)
                return qT, qTh, expn

            cur = head(0)
            for g in range(NQG):          # 8 groups of 4 chunks
                qT, qTh, expn = cur
                for pp in range(2):       # pairs within the group
                    pr = g * 2 + pp       # global pair index
                    pcols = slice(pr * 2 * TK, (pr + 1) * 2 * TK)
                    # ---- ctx fp8 pair [128, 8 dt, 2 hi/lo, 512 tok] ----
                    ctx_sb = wkp.tile([128, 8, 2, 2 * TK], FP8, tag="ctx")
                    for s in range(2):
                        nc.sync.dma_start(
                            out=ctx_sb[:, :, s, :],
                            in_=ctx8[s, :, :].rearrange(
                                "(dt p) t -> p dt t", p=128)[:, :, pcols])

                    # ---- k projection (fp8 DR, act-compensated) ----
                    # kraw [128 = head-pair dh, 4 hp, 512] bf16
                    kraw = wkp.tile([128, 4, 2 * TK], BF16, tag="kraw")
                    for it in range(4):
                        kps = psp.tile([128, 2 * TK], F32, tag="p2",
                                       name=f"kps{pr}_{it}")
                        for dt in range(8):
                            nc.tensor.matmul(
                                kps, wk_sb[:, dt, :, it * 128:(it + 1) * 128],
                                ctx_sb[:, dt, :, :],
                                start=(dt == 0), stop=(dt == 7),
                                perf_mode=DR)
                        nc.scalar.copy(kraw[:, it, :], kps)
                    # ---- rope-k on DVE: kT2 = cos*k + sin_shift*k_shift ----
                    t1k = wkp.tile([128, 4, 2 * TK], BF16, tag="t1k")
                    nc.vector.tensor_mul(
                        t1k[:, :, :].rearrange("p h (r c) -> p h r c", r=4),
                        kraw[:, :, :].rearrange("p h (r c) -> p h r c", r=4),
                        cosk_sb.unsqueeze(1).unsqueeze(2)
                        .broadcast_to((128, 4, 4, 128)))
                    t2k = wkp.tile([128, 4, 2 * TK], BF16, tag="t2k")
                    for (dst, src) in ((0, 32), (32, 0), (64, 96), (96, 64)):
                        nc.vector.tensor_mul(
                            t2k[dst:dst + 32, :, :].rearrange(
                                "p h (r c) -> p h r c", r=4),
                            kraw[src:src + 32, :, :].rearrange(
                                "p h (r c) -> p h r c", r=4),
                            sink2_sb[src:src + 32, :].unsqueeze(1).unsqueeze(2)
                            .broadcast_to((32, 4, 4, 128)))
                    kT2 = wkp.tile([128, 4, 2 * TK], BF16, tag="kT2")
                    nc.vector.tensor_add(kT2, t1k, t2k)
                    kT2h = wkp.tile([64, 4, 2 * TK], BF16, tag="kT2h")
                    nc.vector.tensor_copy(kT2h, kT2[64:128, :, :])

                    if stage <= 1:
                        dbg = wkp.tile([128, DIM], F32, tag="out_sb")
                        nc.vector.tensor_copy(dbg[:, 0:512], kT2[:, 0, :])
                        nc.vector.memset(dbg[:, 512:], 0.0)
                        nc.sync.dma_start(
                            out=out[pr * 2 * CS:(pr + 1) * 2 * CS, :], in_=dbg)
                        continue

                    # ---- per-pair o psum [128 = 2 chunks, 8 h, 128] ----
                    ops_ = None
                    if stage in (3, 4):
                        ops_ = psp.tile([128, 8, 128], F32, tag="p4",
                                        name=f"ops{pr}")
                    # ---- v projections for BOTH chunks first: keeps the PE
                    # busy while the DVE rope chain for this pair runs ----
                    v_augs = []
                    for sub in range(2):
                        c = pr * 2 + sub            # global chunk
                        v_aug = wkp.tile([128, 2, 8, 65], BF16, tag="v_aug", bufs=3)
                        if stage == 17:      # skip v-proj, dummy v_aug
                            nc.vector.memset(v_aug[:, :, :, :], 1.0)
                        for tg in range(2 if stage != 17 else 0):
                            vps = psp.tile([128, INNER], F32, tag="pv",
                                           name=f"vps{c}_{tg}")
                            tsl = slice(sub * TK + tg * 128,
                                        sub * TK + (tg + 1) * 128)
                            for dt in range(8):
                                for w in range(2):
                                    nc.tensor.matmul(
                                        vps,
                                        ctx_sb[:, dt, :, tsl],
                                        wv_sb[:, dt, w, :, :],
                                        start=(dt == 0 and w == 0),
                                        stop=(dt == 7 and w == 1),
                                        perf_mode=DR)
                            nc.scalar.activation(
                                v_aug[:, tg, :, 0:64],
                                vps[:, :].rearrange("p (h w) -> p h w", h=8),
                                COPY, scale=PSUM_SC)
                        nc.gpsimd.memset(v_aug[:, :, :, 64:65], 1.0)
                        v_augs.append(v_aug)

                    for sub in range(2):
                        c = pr * 2 + sub            # global chunk
                        cc = pp * 2 + sub           # chunk within group
                        v_aug = v_augs[sub]
                        if stage == 15:      # v-proj only
                            if sub == 1:
                                continue
                            dbg = wkp.tile([128, DIM], F32, tag="out_sb")
                            nc.vector.tensor_copy(
                                dbg[:, 0:512].rearrange(
                                    "p (h w) -> p h w", h=8),
                                v_aug[:, 0, :, 0:64])
                            nc.vector.memset(dbg[:, 512:], 0.0)
                            nc.sync.dma_start(
                                out=out[pr * 2 * CS:(pr + 1) * 2 * CS, :],
                                in_=dbg)
                            continue

                        # ---- sim [128 j, 2 jg, (h, i)] ----
                        sps = psp.tile([128, 2, INNER], F32, tag="p4",
                                       name=f"sps{c}")
                        for h in range(H):
                            kt = kT2 if h % 2 == 0 else kT2h
                            qt = qT if h % 2 == 0 else qTh
                            for jg in range(2):
                                jsl = slice(sub * TK + jg * 128,
                                            sub * TK + (jg + 1) * 128)
                                nc.tensor.matmul(
                                    sps[:, jg, h * 64:(h + 1) * 64],
                                    kt[0:64, h // 2, jsl],
                                    qt[0:64, h // 2,
                                       cc * CS:(cc + 1) * CS],
                                    start=True, stop=True)
                        expT = wkp.tile([128, 2, INNER], BF16, tag="expT")
                        nc.scalar.activation(expT, sps, EXP)
                        if stage <= 2 or stage == 17:
                            if sub == 1:
                                continue
                            dbg = wkp.tile([128, DIM], F32, tag="out_sb")
                            nc.vector.tensor_copy(dbg[:, 0:512], expT[:, 0, :])
                            nc.vector.tensor_copy(dbg[:, 512:], v_aug[
                                :, :, :, :].rearrange("p a h w -> p (a h w)")
                                [:, 0:512])
                            nc.sync.dma_start(
                                out=out[pr * 2 * CS:(pr + 1) * 2 * CS, :],
                                in_=dbg)
                            continue

                        # ---- o matmuls into pair psum halves ----
                        # one start per 2KB psum bank region (heads 0-3 / 4-7)
                        for h in range(H):
                            dst = ops_[sub * 64:(sub + 1) * 64, h, 0:65]
                            for jg in range(2):
                                nc.tensor.matmul(
                                    dst, expT[:, jg, h * 64:(h + 1) * 64],
                                    v_aug[:, jg, h, :],
                                    start=(h % 4 == 0 and jg == 0),
                                    stop=False, skip_group_check=True)
                        for h in range(H):
                            nc.tensor.matmul(
                                ops_[sub * 64:(sub + 1) * 64, h, 0:65],
                                expn[:, h // 2, cc * CS:(cc + 1) * CS],
                                nullv_sb[:, h // 2, h % 2, :],
                                start=False, stop=(h % 4 == 3),
                                skip_group_check=True)

                    if stage <= 2 or stage in (15, 17):
                        continue
                    # ---- normalize pair on DVE ----
                    rcol = wkp.tile([128, 8], F32, tag="rcol")
                    nc.vector.reciprocal(rcol, ops_[:, :, 64])
                    o_pair = wkp.tile([128, 8, 64], BF16, tag="o_pair")
                    nc.vector.tensor_mul(
                        o_pair, ops_[:, :, 0:64],
                        rcol.unsqueeze(2).broadcast_to((128, 8, 64)))

                    if stage <= 3:
                        dbg = wkp.tile([128, DIM], F32, tag="out_sb")
                        nc.vector.tensor_copy(dbg[:, 0:512], o_pair[
                            :, :, :].rearrange("p h w -> p (h w)"))
                        nc.vector.memset(dbg[:, 512:], 0.0)
                        nc.sync.dma_start(
                            out=out[pr * 2 * CS:(pr + 1) * 2 * CS, :], in_=dbg)
                        continue
                    # ---- transpose -> oT bf16, out projection ----
                    otr = psp.tile([128, 4, 128], BF16, tag="p2",
                                   name=f"otr{pr}")
                    for et in range(4):
                        nc.tensor.transpose(
                            otr[:, et, :], o_pair[:, 2 * et:2 * et + 2, :],
                            ident)
                    oT = wkp.tile([128, 4, 128], BF16, tag="oT")
                    nc.scalar.copy(oT, otr)
                    outps = psp.tile([128, DIM], F32, tag="p4",
                                     name=f"outps{pr}")
                    for co in range(2):
                        for et in range(4):
                            nc.tensor.matmul(
                                outps[:, co * 512:(co + 1) * 512],
                                oT[:, et, :],
                                wo_sb[:, et, co * 512:(co + 1) * 512],
                                start=(et == 0), stop=(et == 3))
                    out_sb = wkp.tile([128, DIM], F32, tag="out_sb")
                    nc.vector.tensor_add(out_sb, outps, bo_sb)
                    nc.sync.dma_start(
                        out=out[pr * 2 * CS:(pr + 1) * 2 * CS, :], in_=out_sb)
                    if pp == 0 and g + 1 < NQG:
                        cur = head(g + 1)

    nc.compile()
    return nc


_CACHED_NC = None


def _get_nc():
    global _CACHED_NC
    if _CACHED_NC is None:
        _CACHED_NC = _build_bass()
    return _CACHED_NC


def _prep_shared(Wq, Wk, Wv, Wo, bo, null_k, null_v, q_pos_emb, k_pos_emb):
    wq_h = np.ascontiguousarray(
        (Wq * SCALE).reshape(8, 128, INNER).transpose(1, 0, 2))

    wk_s = (Wk * (2.0 ** SC_WK)).astype(NPF8)
    wk_r = np.ascontiguousarray(
        wk_s.reshape(8, 128, INNER).transpose(1, 0, 2))
    wk_h = np.ascontiguousarray(
        np.broadcast_to(wk_r[:, :, None, :], (128, 8, 2, INNER)))

    wv32 = Wv * (2.0 ** SC_WV)
    wv_hi = wv32.astype(NPF8)
    wv_lo = (wv32 - wv_hi.astype(np.float32)).astype(NPF8)
    wv_h = np.empty((128, 8, 2, 2, INNER), dtype=NPF8)
    for wi, wmat in enumerate((wv_hi, wv_lo)):
        wr = wmat.reshape(8, 128, INNER).transpose(1, 0, 2)
        wv_h[:, :, wi, 0, :] = wr
        wv_h[:, :, wi, 1, :] = wr

    wo_h = np.ascontiguousarray(
        Wo.reshape(4, 128, DIM).transpose(1, 0, 2)).astype(NPBF)

    # rope-k tables [128 = 2x64 dh halves, 128 pos], psum 2^-12 folded in
    kpe = k_pos_emb[0, 0]                       # [128 pos, 64 dh]
    cos64 = (np.cos(kpe.T) * PSUM_SC).astype(np.float32)   # [64 dh, 128 pos]
    sin64 = (np.sin(kpe.T) * PSUM_SC).astype(np.float32)
    cosk_h = np.concatenate([cos64, cos64], axis=0).astype(NPBF)
    # sin table pre-shifted+signed: reading at src partition yields the value
    # for the dst partition. dst 0:32 <- src 32:64 with -sin[dst]; dst 32:64
    # <- src 0:32 with +sin[dst].
    sin2 = np.empty((64, 128), np.float32)
    sin2[32:64] = -sin64[0:32]
    sin2[0:32] = sin64[32:64]
    sink2_h = np.concatenate([sin2, sin2], axis=0).astype(NPBF)

    qpe63 = q_pos_emb[0, 0, CP]                 # [64]
    cos_q0 = np.cos(qpe63)[:, None].astype(np.float32)
    sgn = np.where(np.arange(64) < 32, -1.0, 1.0)
    sin_q0 = (np.sin(qpe63) * sgn)[:, None].astype(np.float32)
    sp = np.empty_like(sin_q0)
    sp[0:32] = sin_q0[32:64]
    sp[32:64] = sin_q0[0:32]
    cos_q0 = np.concatenate([cos_q0, cos_q0], axis=0)      # [128, 1]
    sin_q0s = np.concatenate([sp, sp], axis=0)

    nk = null_k.reshape(8, 64)                  # [h, dh]
    nullk_h = np.zeros((128, 4, 2), np.float32)
    for h in range(8):
        hb = (h % 2) * 64
        nullk_h[hb:hb + 64, h // 2, h % 2] = nk[h]
    nv = null_v.reshape(8, 64)
    nullv_h = np.zeros((2, 4, 2, 65), np.float32)
    for h in range(8):
        nullv_h[h % 2, h // 2, h % 2, 0:64] = nv[h]
        nullv_h[h % 2, h // 2, h % 2, 64] = 1.0

    return {
        "wq": wq_h, "wk8": wk_h, "wv8": wv_h, "wo": wo_h, "bo": bo,
        "cosk": cosk_h, "sink2": sink2_h,
        "cos_q0": cos_q0, "sin_q0s": sin_q0s,
        "nullk2": nullk_h.astype(NPBF), "nullv2": nullv_h.astype(NPBF),
    }


def kernel(x, context, q_pos_emb, k_pos_emb, Wq, Wk, Wv, Wo, bo, null_k, null_v):
    x = np.asarray(x, dtype=np.float32)
    context = np.asarray(context, dtype=np.float32)
    q_pos_emb = np.asarray(q_pos_emb, dtype=np.float32)
    k_pos_emb = np.asarray(k_pos_emb, dtype=np.float32)
    Wq = np.asarray(Wq, dtype=np.float32)
    Wk = np.asarray(Wk, dtype=np.float32)
    Wv = np.asarray(Wv, dtype=np.float32)
    Wo = np.asarray(Wo, dtype=np.float32)
    bo = np.asarray(bo, dtype=np.float32)
    null_k = np.asarray(null_k, dtype=np.float32)
    null_v = np.asarray(null_v, dtype=np.float32)

    xs = np.zeros_like(x)
    xs[:, : N - CP] = x[:, CP:]
    xc = xs.reshape(BK, CS, DIM)
    ctx = context.reshape(BK, TK, DIM)

    shared = _prep_shared(Wq, Wk, Wv, Wo, bo, null_k, null_v,
                          q_pos_emb, k_pos_emb)

    in_maps = []
    for c in range(N_CORES):
        sl = slice(c * CPC, (c + 1) * CPC)
        xT_c = np.ascontiguousarray(xc[sl].reshape(TQ, DIM).T)
        ctxT_c = np.ascontiguousarray(
            ctx[sl].reshape(TCTX, DIM).T) * (2.0 ** SC_CTX)
        hi = ctxT_c.astype(NPF8)
        lo = (ctxT_c - hi.astype(np.float32)).astype(NPF8)
        ctx8_c = np.stack([hi, lo], axis=0)     # [2, DIM, TCTX]
        in_maps.append({"xT": xT_c, "ctx8": ctx8_c, **shared})

    nc = _get_nc()
    res = run_bass_kernel_spmd(nc, in_maps, core_ids=list(range(N_CORES)))

    out_full = np.concatenate([res.results[c]["out"] for c in range(N_CORES)],
                              axis=0)                      # [BK*CS, DIM]
    o = out_full.reshape(B, K_CHUNKS * CS, DIM)
    final = np.concatenate(
        [np.zeros((B, CP, DIM), np.float32), o[:, : K_CHUNKS * CS - CP]],
        axis=1)
    return final


# revision 25
# speedup vs baseline: 1.5560x; 1.1055x over previous
"""Trainium2 Bass kernel for nn_ChunkedCrossAttention_85907935855128.

Self-contained: hardcodes shapes/sharding. Accepts FULL inputs, returns FULL
output. Shards the fused (b*k_chunks) chunk axis across 8 NeuronCores.

v2 dataflow per core (32 chunks):
  - k-projection: fp8e4m3 DoubleRow, activation hi+lo error compensation
    (ctx quantized on host to hi+residual fp8 pair; Wk single fp8).
  - v-projection: fully-compensated fp8 DoubleRow ((hi+lo)@(Whi+Wlo)); same
    cost as bf16 but lets ctx live in SBUF as the fp8 pair only.
  - q-projection: fp32r. out-projection: bf16.
  - rope-k on DVE (partition-shifted sin muls, psum scale folded in tables),
    no PE perm matmul.
  - attention: pair-stacked o psum (2 chunks on partition halves), exp bf16
    on ACT, softmax sum via ones column, null-k/v via zero-masked K=2 mms.
"""
import os
# bass2jax executes via the axon PJRT platform; a CPU pin would hide the cores.
if os.environ.get("JAX_PLATFORMS", "") in ("cpu",):
    del os.environ["JAX_PLATFORMS"]

import numpy as np
import ml_dtypes

import concourse.bacc as bacc
import concourse.bass as bass
import concourse.mybir as mybir
import concourse.tile as tile
from concourse.bass_utils import run_bass_kernel_spmd
from concourse.masks import make_identity

F32 = mybir.dt.float32
F32R = mybir.dt.float32r
BF16 = mybir.dt.bfloat16
FP8 = mybir.dt.float8e4
NPF8 = ml_dtypes.float8_e4m3
NPBF = ml_dtypes.bfloat16
DR = mybir.MatmulPerfMode.DoubleRow
EXP = mybir.ActivationFunctionType.Exp
COPY = mybir.ActivationFunctionType.Copy

CS, CP, H, DH = 64, 63, 8, 64
SCALE = DH ** -0.5
N_CORES = 8
B, N, DIM = 4, 4096, 1024
K_CHUNKS, R, RLEN = 64, 2, 128
TK = R * RLEN                 # 256 ctx tokens / chunk
BK = B * K_CHUNKS             # 256 chunks
CPC = BK // N_CORES           # 32 chunks / core
TQ = CPC * CS                 # 2048 q tokens / core
TCTX = CPC * TK               # 8192 ctx tokens / core
INNER = H * DH                # 512
QG = 4                        # chunks per q-projection group
NQG = CPC // QG               # 8 q groups / core

SC_CTX = 3                    # ctx * 2^3 before fp8
SC_WK = 9                     # Wk * 2^9
SC_WV = 9
PSUM_SC = 2.0 ** -(SC_CTX + SC_WK)   # folded into rope tables / v evac


def _build_bass(num_devices=N_CORES, stage=4):
    nc = bacc.Bacc("TRN2", target_bir_lowering=False, debug=False,
                   num_devices=num_devices)

    xT = nc.dram_tensor("xT", (DIM, TQ), F32, kind="ExternalInput")
    ctx8 = nc.dram_tensor("ctx8", (2, DIM, TCTX), FP8, kind="ExternalInput")
    wq = nc.dram_tensor("wq", (128, 8, INNER), F32, kind="ExternalInput")
    wk8 = nc.dram_tensor("wk8", (128, 8, 2, INNER), FP8, kind="ExternalInput")
    wv8 = nc.dram_tensor("wv8", (128, 8, 2, 2, INNER), FP8, kind="ExternalInput")
    wo = nc.dram_tensor("wo", (128, 4, DIM), BF16, kind="ExternalInput")
    bo = nc.dram_tensor("bo", (DIM,), F32, kind="ExternalInput")
    cosk = nc.dram_tensor("cosk", (128, 128), BF16, kind="ExternalInput")
    sink2 = nc.dram_tensor("sink2", (128, 128), BF16, kind="ExternalInput")
    cos_q0 = nc.dram_tensor("cos_q0", (128, 1), F32, kind="ExternalInput")
    sin_q0s = nc.dram_tensor("sin_q0s", (128, 1), F32, kind="ExternalInput")
    nullk2 = nc.dram_tensor("nullk2", (128, 4, 2), BF16, kind="ExternalInput")
    nullv2 = nc.dram_tensor("nullv2", (2, 4, 2, 65), BF16, kind="ExternalInput")
    out = nc.dram_tensor("out", (TQ, DIM), F32, kind="ExternalOutput")

    with tile.TileContext(nc) as tc:
        with tc.tile_pool(name="consts", bufs=1) as cp_, \
             tc.tile_pool(name="wk", bufs=2) as wkp, \
             tc.tile_pool(name="ps", bufs=2, space="PSUM") as psp:

            # ---- constants (ordered by first use) ----
            wq_sb = cp_.tile([128, 8, INNER], F32R)
            nc.sync.dma_start(out=wq_sb, in_=wq[:, :, :].bitcast(F32R))
            wk_sb = cp_.tile([128, 8, 2, INNER], FP8)
            nc.sync.dma_start(out=wk_sb, in_=wk8[:, :, :, :])
            cosk_sb = cp_.tile([128, 128], BF16)
            nc.sync.dma_start(out=cosk_sb, in_=cosk[:, :])
            sink2_sb = cp_.tile([128, 128], BF16)
            nc.sync.dma_start(out=sink2_sb, in_=sink2[:, :])
            cosq_sb = cp_.tile([128, 1], F32)
            nc.sync.dma_start(out=cosq_sb, in_=cos_q0[:, :])
            sinq_sb = cp_.tile([128, 1], F32)
            nc.sync.dma_start(out=sinq_sb, in_=sin_q0s[:, :])
            nullk_sb = cp_.tile([128, 4, 2], BF16)
            nc.sync.dma_start(out=nullk_sb, in_=nullk2[:, :, :])
            nullv_sb = cp_.tile([2, 4, 2, 65], BF16)
            nc.sync.dma_start(out=nullv_sb, in_=nullv2[:, :, :, :])
            ident = cp_.tile([128, 128], BF16)
            make_identity(nc, ident)
            wv_sb = cp_.tile([128, 8, 2, 2, INNER], FP8)
            nc.sync.dma_start(out=wv_sb, in_=wv8[:, :, :, :, :])
            wo_sb = cp_.tile([128, 4, DIM], BF16)
            nc.sync.dma_start(out=wo_sb, in_=wo[:, :, :])
            bo_sb = cp_.tile([128, DIM], F32)
            nc.sync.dma_start(out=bo_sb, in_=bass.AP(
                tensor=bo, offset=0, ap=[[0, 128], [1, DIM]]))

            def head(g):
                """q-projection + rope-q + null-k sims for one group."""
                gcols = slice(g * QG * CS, (g + 1) * QG * CS)
                # ---- q projection (fp32r): qps [128=2 heads, 2, 256] x2 ----
                xT_sb = wkp.tile([128, 8, QG * CS], F32R, tag="xT")
                nc.sync.dma_start(out=xT_sb, in_=xT[:, :].rearrange(
                    "(dt p) t -> p dt t", p=128)[:, :, gcols].bitcast(F32R))
                qps = [psp.tile([128, 2, QG * CS], F32, tag="p2",
                                name=f"qps{g}_{i}") for i in range(2)]
                for it in range(4):
                    for dt in range(8):
                        nc.tensor.matmul(
                            qps[it // 2][:, it % 2, :],
                            wq_sb[:, dt, it * 128:(it + 1) * 128],
                            xT_sb[:, dt, :],
                            start=(dt == 0), stop=(dt == 7))
                # qT [128 = head-pair dh, 4 hp, 256] bf16
                qT = wkp.tile([128, 4, QG * CS], BF16, tag="qT")
                for it in range(4):
                    nc.scalar.copy(qT[:, it, :], qps[it // 2][:, it % 2, :])
                # rope-q: fix token 0 of each chunk (cols ::CS), both halves
                qcols = qT[:, :, :].rearrange(
                    "p h (c w) -> p h c w", w=CS)[:, :, :, 0]   # [128, 4, QG]
                t1q = wkp.tile([128, 4, QG], BF16, tag="t1q")
                nc.vector.tensor_mul(
                    t1q, qcols, cosq_sb.unsqueeze(2).broadcast_to((128, 4, QG)))
                t2q = wkp.tile([128, 4, QG], BF16, tag="t2q")
                for (dst, src) in ((0, 32), (32, 0), (64, 96), (96, 64)):
                    nc.vector.tensor_mul(
                        t2q[dst:dst + 32, :, :],
                        qT[:, :, :].rearrange(
                            "p h (c w) -> p h c w", w=CS)[src:src + 32, :, :, 0],
                        sinq_sb[src:src + 32, :].unsqueeze(2)
                        .broadcast_to((32, 4, QG)))
                nc.vector.tensor_add(qcols, t1q, t2q)
                # odd heads live on partitions 64-127; PE operands must sit at
                # base 0 on hw, so keep a base-0 copy for the sim matmuls
                qTh = wkp.tile([64, 4, QG * CS], BF16, tag="qTh")
                nc.vector.tensor_copy(qTh, qT[64:128, :, :])

                # ---- null-k sims -> expn bf16 [2, 4 hp, 256] ----
                expn = wkp.tile([2, 4, QG * CS], BF16, tag="expn")
                for half in range(2):
                    nps = psp.tile([2, 2, QG * CS], F32, tag="p2",
                                   name=f"nps{g}_{half}")
                    for hh in range(2):
                        hp = half * 2 + hh
                        nc.tensor.matmul(
                            nps[:, hh, :], nullk_sb[:, hp, :], qT[:, hp, :],
                            start=True, stop=True)
                    nc.scalar.activation(
                        expn[:, half * 2:(half + 1) * 2, :], nps, EXP)
                return qT, qTh, expn

            cur = head(0)
            for g in range(NQG):          # 8 groups of 4 chunks
                qT, qTh, expn = cur
                for pp in range(2):       # pairs within the group
                    pr = g * 2 + pp       # global pair index
                    pcols = slice(pr * 2 * TK, (pr + 1) * 2 * TK)
                    # ---- ctx fp8 pair [128, 8 dt, 2 hi/lo, 512 tok] ----
                    ctx_sb = wkp.tile([128, 8, 2, 2 * TK], FP8, tag="ctx")
                    for s in range(2):
                        nc.sync.dma_start(
                            out=ctx_sb[:, :, s, :],
                            in_=ctx8[s, :, :].rearrange(
                                "(dt p) t -> p dt t", p=128)[:, :, pcols])

                    # ---- k projection (fp8 DR, act-compensated) ----
                    # kraw [128 = head-pair dh, 4 hp, 512] bf16; rope each
                    # head-pair as soon as its psum is evacuated so the DVE
                    # chain overlaps the remaining PE matmuls
                    kraw = wkp.tile([128, 4, 2 * TK], BF16, tag="kraw")
                    t1k = wkp.tile([128, 4, 2 * TK], BF16, tag="t1k")
                    t2k = wkp.tile([128, 4, 2 * TK], BF16, tag="t2k")
                    kT2 = wkp.tile([128, 4, 2 * TK], BF16, tag="kT2")
                    kT2h = wkp.tile([64, 4, 2 * TK], BF16, tag="kT2h")
                    for it in range(4):
                        kps = psp.tile([128, 2 * TK], F32, tag="p2",
                                       name=f"kps{pr}_{it}")
                        for dt in range(8):
                            nc.tensor.matmul(
                                kps, wk_sb[:, dt, :, it * 128:(it + 1) * 128],
                                ctx_sb[:, dt, :, :],
                                start=(dt == 0), stop=(dt == 7),
                                perf_mode=DR)
                        nc.scalar.copy(kraw[:, it, :], kps)
                        its = slice(it, it + 1)
                        nc.vector.tensor_mul(
                            t1k[:, its, :].rearrange("p h (r c) -> p h r c", r=4),
                            kraw[:, its, :].rearrange("p h (r c) -> p h r c", r=4),
                            cosk_sb.unsqueeze(1).unsqueeze(2)
                            .broadcast_to((128, 1, 4, 128)))
                        for (dst, srcp) in ((0, 32), (32, 0), (64, 96), (96, 64)):
                            nc.vector.tensor_mul(
                                t2k[dst:dst + 32, its, :].rearrange(
                                    "p h (r c) -> p h r c", r=4),
                                kraw[srcp:srcp + 32, its, :].rearrange(
                                    "p h (r c) -> p h r c", r=4),
                                sink2_sb[srcp:srcp + 32, :].unsqueeze(1)
                                .unsqueeze(2).broadcast_to((32, 1, 4, 128)))
                        nc.vector.tensor_add(kT2[:, its, :], t1k[:, its, :],
                                             t2k[:, its, :])
                        nc.vector.tensor_copy(kT2h[:, its, :],
                                              kT2[64:128, its, :])

                    if stage <= 1:
                        dbg = wkp.tile([128, DIM], F32, tag="out_sb")
                        nc.vector.tensor_copy(dbg[:, 0:512], kT2[:, 0, :])
                        nc.vector.memset(dbg[:, 512:], 0.0)
                        nc.sync.dma_start(
                            out=out[pr * 2 * CS:(pr + 1) * 2 * CS, :], in_=dbg)
                        continue

                    # ---- per-pair o psum [128 = 2 chunks, 8 h, 128] ----
                    ops_ = None
                    if stage in (3, 4):
                        ops_ = psp.tile([128, 8, 128], F32, tag="p4",
                                        name=f"ops{pr}")
                    # ---- v projections for BOTH chunks first: keeps the PE
                    # busy while the DVE rope chain for this pair runs ----
                    v_augs = []
                    for sub in range(2):
                        c = pr * 2 + sub            # global chunk
                        v_aug = wkp.tile([128, 2, 8, 65], BF16, tag="v_aug", bufs=3)
                        if stage == 17:      # skip v-proj, dummy v_aug
                            nc.vector.memset(v_aug[:, :, :, :], 1.0)
                        for tg in range(2 if stage != 17 else 0):
                            vps = psp.tile([128, INNER], F32, tag="pv",
                                           name=f"vps{c}_{tg}")
                            tsl = slice(sub * TK + tg * 128,
                                        sub * TK + (tg + 1) * 128)
                            for dt in range(8):
                                for w in range(2):
                                    nc.tensor.matmul(
                                        vps,
                                        ctx_sb[:, dt, :, tsl],
                                        wv_sb[:, dt, w, :, :],
                                        start=(dt == 0 and w == 0),
                                        stop=(dt == 7 and w == 1),
                                        perf_mode=DR)
                            nc.scalar.activation(
                                v_aug[:, tg, :, 0:64],
                                vps[:, :].rearrange("p (h w) -> p h w", h=8),
                                COPY, scale=PSUM_SC)
                        nc.gpsimd.memset(v_aug[:, :, :, 64:65], 1.0)
                        v_augs.append(v_aug)

                    for sub in range(2):
                        c = pr * 2 + sub            # global chunk
                        cc = pp * 2 + sub           # chunk within group
                        v_aug = v_augs[sub]
                        if stage == 15:      # v-proj only
                            if sub == 1:
                                continue
                            dbg = wkp.tile([128, DIM], F32, tag="out_sb")
                            nc.vector.tensor_copy(
                                dbg[:, 0:512].rearrange(
                                    "p (h w) -> p h w", h=8),
                                v_aug[:, 0, :, 0:64])
                            nc.vector.memset(dbg[:, 512:], 0.0)
                            nc.sync.dma_start(
                                out=out[pr * 2 * CS:(pr + 1) * 2 * CS, :],
                                in_=dbg)
                            continue

                        # ---- sim [128 j, 2 jg, (h, i)] ----
                        sps = psp.tile([128, 2, INNER], F32, tag="p4",
                                       name=f"sps{c}")
                        for h in range(H):
                            kt = kT2 if h % 2 == 0 else kT2h
                            qt = qT if h % 2 == 0 else qTh
                            for jg in range(2):
                                jsl = slice(sub * TK + jg * 128,
                                            sub * TK + (jg + 1) * 128)
                                nc.tensor.matmul(
                                    sps[:, jg, h * 64:(h + 1) * 64],
                                    kt[0:64, h // 2, jsl],
                                    qt[0:64, h // 2,
                                       cc * CS:(cc + 1) * CS],
                                    start=True, stop=True)
                        expT = wkp.tile([128, 2, INNER], BF16, tag="expT")
                        nc.scalar.activation(expT, sps, EXP)
                        if stage <= 2 or stage == 17:
                            if sub == 1:
                                continue
                            dbg = wkp.tile([128, DIM], F32, tag="out_sb")
                            nc.vector.tensor_copy(dbg[:, 0:512], expT[:, 0, :])
                            nc.vector.tensor_copy(dbg[:, 512:], v_aug[
                                :, :, :, :].rearrange("p a h w -> p (a h w)")
                                [:, 0:512])
                            nc.sync.dma_start(
                                out=out[pr * 2 * CS:(pr + 1) * 2 * CS, :],
                                in_=dbg)
                            continue

                        # ---- o matmuls into pair psum halves ----
                        # one start per 2KB psum bank region (heads 0-3 / 4-7)
                        for h in range(H):
                            dst = ops_[sub * 64:(sub + 1) * 64, h, 0:65]
                            for jg in range(2):
                                nc.tensor.matmul(
                                    dst, expT[:, jg, h * 64:(h + 1) * 64],
                                    v_aug[:, jg, h, :],
                                    start=(h % 4 == 0 and jg == 0),
                                    stop=False, skip_group_check=True)
                        for h in range(H):
                            nc.tensor.matmul(
                                ops_[sub * 64:(sub + 1) * 64, h, 0:65],
                                expn[:, h // 2, cc * CS:(cc + 1) * CS],
                                nullv_sb[:, h // 2, h % 2, :],
                                start=False, stop=(h % 4 == 3),
                                skip_group_check=True)

                    if stage <= 2 or stage in (15, 17):
                        continue
                    # ---- normalize pair on DVE ----
                    rcol = wkp.tile([128, 8], F32, tag="rcol")
                    nc.vector.reciprocal(rcol, ops_[:, :, 64])
                    o_pair = wkp.tile([128, 8, 64], BF16, tag="o_pair")
                    nc.vector.tensor_mul(
                        o_pair, ops_[:, :, 0:64],
                        rcol.unsqueeze(2).broadcast_to((128, 8, 64)))

                    if stage <= 3:
                        dbg = wkp.tile([128, DIM], F32, tag="out_sb")
                        nc.vector.tensor_copy(dbg[:, 0:512], o_pair[
                            :, :, :].rearrange("p h w -> p (h w)"))
                        nc.vector.memset(dbg[:, 512:], 0.0)
                        nc.sync.dma_start(
                            out=out[pr * 2 * CS:(pr + 1) * 2 * CS, :], in_=dbg)
                        continue
                    # ---- transpose -> oT bf16, out projection ----
                    otr = psp.tile([128, 4, 128], BF16, tag="p2",
                                   name=f"otr{pr}")
                    for et in range(4):
                        nc.tensor.transpose(
                            otr[:, et, :], o_pair[:, 2 * et:2 * et + 2, :],
                            ident)
                    oT = wkp.tile([128, 4, 128], BF16, tag="oT")
                    nc.scalar.copy(oT, otr)
                    outps = psp.tile([128, DIM], F32, tag="p4",
                                     name=f"outps{pr}")
                    for co in range(2):
                        for et in range(4):
                            nc.tensor.matmul(
                                outps[:, co * 512:(co + 1) * 512],
                                oT[:, et, :],
                                wo_sb[:, et, co * 512:(co + 1) * 512],
                                start=(et == 0), stop=(et == 3))
                    out_sb = wkp.tile([128, DIM], F32, tag="out_sb")
                    nc.vector.tensor_add(out_sb, outps, bo_sb)
                    nc.sync.dma_start(
                        out=out[pr * 2 * CS:(pr + 1) * 2 * CS, :], in_=out_sb)
                    if pp == 0 and g + 1 < NQG:
                        cur = head(g + 1)

    nc.compile()
    return nc


_CACHED_NC = None


def _get_nc():
    global _CACHED_NC
    if _CACHED_NC is None:
        _CACHED_NC = _build_bass()
    return _CACHED_NC


def _prep_shared(Wq, Wk, Wv, Wo, bo, null_k, null_v, q_pos_emb, k_pos_emb):
    wq_h = np.ascontiguousarray(
        (Wq * SCALE).reshape(8, 128, INNER).transpose(1, 0, 2))

    wk_s = (Wk * (2.0 ** SC_WK)).astype(NPF8)
    wk_r = np.ascontiguousarray(
        wk_s.reshape(8, 128, INNER).transpose(1, 0, 2))
    wk_h = np.ascontiguousarray(
        np.broadcast_to(wk_r[:, :, None, :], (128, 8, 2, INNER)))

    wv32 = Wv * (2.0 ** SC_WV)
    wv_hi = wv32.astype(NPF8)
    wv_lo = (wv32 - wv_hi.astype(np.float32)).astype(NPF8)
    wv_h = np.empty((128, 8, 2, 2, INNER), dtype=NPF8)
    for wi, wmat in enumerate((wv_hi, wv_lo)):
        wr = wmat.reshape(8, 128, INNER).transpose(1, 0, 2)
        wv_h[:, :, wi, 0, :] = wr
        wv_h[:, :, wi, 1, :] = wr

    wo_h = np.ascontiguousarray(
        Wo.reshape(4, 128, DIM).transpose(1, 0, 2)).astype(NPBF)

    # rope-k tables [128 = 2x64 dh halves, 128 pos], psum 2^-12 folded in
    kpe = k_pos_emb[0, 0]                       # [128 pos, 64 dh]
    cos64 = (np.cos(kpe.T) * PSUM_SC).astype(np.float32)   # [64 dh, 128 pos]
    sin64 = (np.sin(kpe.T) * PSUM_SC).astype(np.float32)
    cosk_h = np.concatenate([cos64, cos64], axis=0).astype(NPBF)
    # sin table pre-shifted+signed: reading at src partition yields the value
    # for the dst partition. dst 0:32 <- src 32:64 with -sin[dst]; dst 32:64
    # <- src 0:32 with +sin[dst].
    sin2 = np.empty((64, 128), np.float32)
    sin2[32:64] = -sin64[0:32]
    sin2[0:32] = sin64[32:64]
    sink2_h = np.concatenate([sin2, sin2], axis=0).astype(NPBF)

    qpe63 = q_pos_emb[0, 0, CP]                 # [64]
    cos_q0 = np.cos(qpe63)[:, None].astype(np.float32)
    sgn = np.where(np.arange(64) < 32, -1.0, 1.0)
    sin_q0 = (np.sin(qpe63) * sgn)[:, None].astype(np.float32)
    sp = np.empty_like(sin_q0)
    sp[0:32] = sin_q0[32:64]
    sp[32:64] = sin_q0[0:32]
    cos_q0 = np.concatenate([cos_q0, cos_q0], axis=0)      # [128, 1]
    sin_q0s = np.concatenate([sp, sp], axis=0)

    nk = null_k.reshape(8, 64)                  # [h, dh]
    nullk_h = np.zeros((128, 4, 2), np.float32)
    for h in range(8):
        hb = (h % 2) * 64
        nullk_h[hb:hb + 64, h // 2, h % 2] = nk[h]
    nv = null_v.reshape(8, 64)
    nullv_h = np.zeros((2, 4, 2, 65), np.float32)
    for h in range(8):
        nullv_h[h % 2, h // 2, h % 2, 0:64] = nv[h]
        nullv_h[h % 2, h // 2, h % 2, 64] = 1.0

    return {
        "wq": wq_h, "wk8": wk_h, "wv8": wv_h, "wo": wo_h, "bo": bo,
        "cosk": cosk_h, "sink2": sink2_h,
        "cos_q0": cos_q0, "sin_q0s": sin_q0s,
        "nullk2": nullk_h.astype(NPBF), "nullv2": nullv_h.astype(NPBF),
    }


def kernel(x, context, q_pos_emb, k_pos_emb, Wq, Wk, Wv, Wo, bo, null_k, null_v):
    x = np.asarray(x, dtype=np.float32)
    context = np.asarray(context, dtype=np.float32)
    q_pos_emb = np.asarray(q_pos_emb, dtype=np.float32)
    k_pos_emb = np.asarray(k_pos_emb, dtype=np.float32)
    Wq = np.asarray(Wq, dtype=np.float32)
    Wk = np.asarray(Wk, dtype=np.float32)
    Wv = np.asarray(Wv, dtype=np.float32)
    Wo = np.asarray(Wo, dtype=np.float32)
    bo = np.asarray(bo, dtype=np.float32)
    null_k = np.asarray(null_k, dtype=np.float32)
    null_v = np.asarray(null_v, dtype=np.float32)

    xs = np.zeros_like(x)
    xs[:, : N - CP] = x[:, CP:]
    xc = xs.reshape(BK, CS, DIM)
    ctx = context.reshape(BK, TK, DIM)

    shared = _prep_shared(Wq, Wk, Wv, Wo, bo, null_k, null_v,
                          q_pos_emb, k_pos_emb)

    in_maps = []
    for c in range(N_CORES):
        sl = slice(c * CPC, (c + 1) * CPC)
        xT_c = np.ascontiguousarray(xc[sl].reshape(TQ, DIM).T)
        ctxT_c = np.ascontiguousarray(
            ctx[sl].reshape(TCTX, DIM).T) * (2.0 ** SC_CTX)
        hi = ctxT_c.astype(NPF8)
        lo = (ctxT_c - hi.astype(np.float32)).astype(NPF8)
        ctx8_c = np.stack([hi, lo], axis=0)     # [2, DIM, TCTX]
        in_maps.append({"xT": xT_c, "ctx8": ctx8_c, **shared})

    nc = _get_nc()
    res = run_bass_kernel_spmd(nc, in_maps, core_ids=list(range(N_CORES)))

    out_full = np.concatenate([res.results[c]["out"] for c in range(N_CORES)],
                              axis=0)                      # [BK*CS, DIM]
    o = out_full.reshape(B, K_CHUNKS * CS, DIM)
    final = np.concatenate(
        [np.zeros((B, CP, DIM), np.float32), o[:, : K_CHUNKS * CS - CP]],
        axis=1)
    return final
